# revision 1
# baseline (speedup 1.0000x reference)
"""Trainium2 Bass kernel for CDMamba ModifiedSRCMLayer (self-contained).

Sharding: 8 cores; core k handles batch k//2 and mamba group-pair k%2
(groups {0,1} or {2,3}). Group outputs are exchanged with a paired
AllGather; the post-stage (gate blend + output projection) is computed
redundantly on both cores of a pair and the host reads even cores.

Selective scan runs on the DVE via tensor_tensor_scan over tiles of
[128 partitions = 2 s-values x 64 d, 512 timesteps]; exp(dt*A) on the
scalar engine with per-partition scale; B/C broadcasts, the s-reduction,
convolutions, and projections on the tensor engine. The backward
direction uses negative-step APs (free reversal).
"""
import sys
import numpy as np

for _p in ("/opt/trn_rl_repo",):
    if _p not in sys.path:
        sys.path.append(_p)

import concourse.bass as bass
import concourse.mybir as mybir
from concourse.bacc import Bacc
from concourse.tile import TileContext
from concourse.bass_types import AP as _AP

# Model dims (hardcoded per the problem spec)
B, C, H, W = 4, 128, 64, 64
L = H * W                      # 4096
G, DM = 4, 32
DI, DS, DC = 64, 16, 4
DTR = 2
OUT = 128
EPS = 1e-5

NCORE = 8
LC = 512                       # time chunk
NCH = L // LC                  # 8
NJ = DS // 2                   # 8 s-tiles (2 s-values per tile)
F32 = mybir.dt.float32
BF = mybir.dt.bfloat16
AF = mybir.ActivationFunctionType
ALU = mybir.AluOpType


def _build_nc():
    nc = Bacc(num_devices=NCORE)

    def inp(name, shape, dt=F32):
        return nc.dram_tensor(name, list(shape), dt, kind="ExternalInput")

    # per-core data
    xpad = inp("xpad", (C, 66 * 66))
    pe_b = inp("pe_b", (C, L))
    # weights (already laid out per core-set on the host)
    w9 = inp("w9", (C, 9 * 128))
    mred1 = inp("mred1", (128, 1))
    onesr = inp("onesr", (1, 128))
    ln_g = inp("ln_g", (128, 1))
    ln_b = inp("ln_b", (128, 1))
    gateWT = inp("gateWT", (128, 128))
    gateb = inp("gateb", (128, 1))
    winTu = inp("winTu", (2, C, DI))    # group-select baked in (zero rows)
    winTz = inp("winTz", (2, C, DI))
    conv4T = inp("conv4T", (2, 2, DC, DI, 128), BF)
    convb = inp("convb", (2, 2, 128, 1))
    dtWT = inp("dtWT", (2, 2, DI, 128), BF)
    dtb = inp("dtb", (2, 2, 128, 1))
    xprojBCT = inp("xprojBCT", (2, 2, DI, 2 * DS), BF)
    A_sc = inp("A_sc", (2, 2, 128, NJ))
    mredM = inp("mredM", (128, DI), BF)
    dsk = inp("dsk", (2, 2, 128, 1))
    selBC = inp("selBC", (NJ, DS, 128), BF)
    woutT = inp("woutT", (128, 2 * DM), BF)
    projT = inp("projT", (128, 128))
    projb = inp("projb", (128, 1))

    xm_loc = nc.dram_tensor("xm_loc", [2 * DM, L], F32)
    bc_dram = nc.dram_tensor("bc_dram", [4, DS, L], BF)
    xm_all = nc.dram_tensor("xm_all", [C, L], F32)
    outp = nc.dram_tensor("outp", [OUT, L], F32, kind="ExternalOutput")

    with TileContext(nc) as tc:
        with (
            tc.tile_pool(name="const", bufs=1) as cp,
            tc.tile_pool(name="big", bufs=1) as bp,
            tc.tile_pool(name="hpool", bufs=2) as hp,
            tc.tile_pool(name="psP", bufs=1, space="PSUM") as psP,
        ):
            # ---- load constants to SBUF ----
            def c_load(ap_dram, shape, nm):
                t = cp.tile(list(shape), F32, name=nm, tag=nm)
                nc.sync.dma_start(t[:], ap_dram)
                return t

            w9_sb = c_load(w9[:], (C, 9 * 128), "w9sb")
            mred1_sb = c_load(mred1[:], (128, 1), "mred1sb")
            onesr_sb = c_load(onesr[:], (1, 128), "onesrsb")
            lng_sb = c_load(ln_g[:], (128, 1), "lngsb")
            lnb_sb = c_load(ln_b[:], (128, 1), "lnbsb")
            gateWT_sb = c_load(gateWT[:], (128, 128), "gateWTsb")
            gateb_sb = c_load(gateb[:], (128, 1), "gatebsb")
            mredM_sb = cp.tile([128, DI], BF, name="mredMsb", tag="mredMsb")
            nc.sync.dma_start(mredM_sb[:], mredM[:])
            woutT_sb = cp.tile([128, 2 * DM], BF, name="woutTsb", tag="woutTsb")
            nc.sync.dma_start(woutT_sb[:], woutT[:])
            projT_sb = c_load(projT[:], (128, 128), "projTsb")
            projb_sb = c_load(projb[:], (128, 1), "projbsb")

            winTu_sb = cp.tile([C, 2 * DI], F32)
            winTz_sb = cp.tile([C, 2 * DI], F32)
            conv4T_sb = cp.tile([128, 16 * 128], BF)
            dtWT_sb = cp.tile([DI, 4 * 128], BF)
            xprojBCT_sb = cp.tile([DI, 4 * 2 * DS], BF)
            asc_sb = cp.tile([128, 4 * NJ], F32)
            selBC_sb = cp.tile([DS, NJ * 128], BF)
            convb_sb = cp.tile([128, 4], F32)
            dtb_sb = cp.tile([128, 4], F32)
            dsk_sb = cp.tile([128, 4], F32)
            eps_sb = cp.tile([1, 1], F32)
            nc.vector.memset(eps_sb[:], EPS)
            for j in range(NJ):
                nc.sync.dma_start(selBC_sb[:, j * 128:(j + 1) * 128], selBC[j])
            for gl in range(2):
                nc.sync.dma_start(winTu_sb[:, gl * DI:(gl + 1) * DI], winTu[gl])
                nc.sync.dma_start(winTz_sb[:, gl * DI:(gl + 1) * DI], winTz[gl])
                for dr in range(2):
                    i4 = gl * 2 + dr
                    for k in range(DC):
                        for hh in range(2):
                            nc.sync.dma_start(
                                conv4T_sb[hh * 64:(hh + 1) * 64,
                                          (i4 * 4 + k) * 128:(i4 * 4 + k + 1) * 128],
                                conv4T[gl, dr, k])
                    nc.sync.dma_start(dtWT_sb[:, i4 * 128:(i4 + 1) * 128], dtWT[gl, dr])
                    nc.sync.dma_start(
                        xprojBCT_sb[:, i4 * 2 * DS:(i4 + 1) * 2 * DS], xprojBCT[gl, dr])
                    nc.sync.dma_start(asc_sb[:, i4 * NJ:(i4 + 1) * NJ], A_sc[gl, dr])
                    nc.sync.dma_start(convb_sb[:, i4:i4 + 1], convb[gl, dr])
                    nc.sync.dma_start(dtb_sb[:, i4:i4 + 1], dtb[gl, dr])
                    nc.sync.dma_start(dsk_sb[:, i4:i4 + 1], dsk[gl, dr])

            # ---- big persistent tiles ----
            xs = bp.tile([C, L], F32)       # post pos-embed input, (c, l) layout
            gate = bp.tile([C, L], F32)
            u_pad = bp.tile([C, L + 6], BF)  # rows [g0 u | g1 u]; 3-zero halo
            zs = bp.tile([C, L], BF)       # silu(z), group-packed rows
            yfb = bp.tile([C, L], BF)      # y_fwd + y_bwd, group-packed rows

            nc.vector.memset(u_pad[:, 0:3], 0.0)
            nc.vector.memset(u_pad[:, L + 3:L + 6], 0.0)

            # ---- Phase A: conv-pos-enc + pos-embed + LN (pass 1), then
            # gate + xz (pass 2) — two passes so ACT table sets batch ----
            with tc.tile_pool(name="pA", bufs=2) as pA:
                xpad_sb = pA.tile([C, 66 * 66], F32, bufs=1)
                nc.sync.dma_start(xpad_sb[:], xpad[:])
                xpad3 = xpad_sb[:].rearrange("p (r q) -> p r q", q=66)
                xnc = pA.tile([C, L], F32, bufs=1)
                for c in range(NCH):
                    cs = slice(c * LC, (c + 1) * LC)
                    pa = psP.tile([128, 8, 64], F32, tag="gen", bufs=2)
                    for tap in range(9):
                        dy, dx = tap // 3, tap % 3
                        nc.tensor.matmul(
                            pa[:],
                            w9_sb[:, tap * 128:(tap + 1) * 128],
                            xpad3[:, c * 8 + dy:c * 8 + dy + 8, dx:dx + 64],
                            start=(tap == 0), stop=(tap == 8))
                    paf = pa[:].rearrange("p a b -> p (a b)")
                    pe_t = pA.tile([128, LC], F32, tag="pe")
                    nc.sync.dma_start(pe_t[:], pe_b[:, cs])
                    nc.vector.tensor_tensor(xs[:, cs], paf, pe_t[:], op=ALU.add)

                    mu = psP.tile([1, LC], F32, tag="gen", bufs=2)
                    nc.tensor.matmul(mu[:], mred1_sb[:], xs[:, cs],
                                     start=True, stop=True)
                    mu_sb = pA.tile([1, LC], F32, tag="musb")
                    nc.scalar.copy(mu_sb[:], mu[:])
                    mub = psP.tile([128, LC], F32, tag="gen", bufs=2)
                    nc.tensor.matmul(mub[:], onesr_sb[:], mu_sb[:],
                                     start=True, stop=True)
                    xc = pA.tile([128, LC], F32, tag="xc")
                    nc.vector.tensor_tensor(xc[:], xs[:, cs], mub[:], op=ALU.subtract)
                    xsq = pA.tile([128, LC], F32, tag="xsq")
                    nc.scalar.square(xsq[:], xc[:])
                    var = psP.tile([1, LC], F32, tag="gen", bufs=2)
                    nc.tensor.matmul(var[:], mred1_sb[:], xsq[:], start=True, stop=True)
                    sd = pA.tile([1, LC], F32, tag="sd")
                    nc.scalar.activation(sd[:], var[:], AF.Sqrt, bias=eps_sb[:, 0:1])
                    rstd = pA.tile([1, LC], F32, tag="rstd")
                    nc.vector.reciprocal(rstd[:], sd[:])
                    rstdb = psP.tile([128, LC], F32, tag="gen", bufs=2)
                    nc.tensor.matmul(rstdb[:], onesr_sb[:], rstd[:],
                                     start=True, stop=True)
                    xng = pA.tile([128, LC], F32, tag="xng")
                    nc.vector.tensor_tensor(xng[:], xc[:], rstdb[:], op=ALU.mult)
                    nc.scalar.activation(xnc[:, cs], xng[:], AF.Identity,
                                         bias=lnb_sb[:, 0:1], scale=lng_sb[:, 0:1])

                for c in range(NCH):
                    cs = slice(c * LC, (c + 1) * LC)
                    gps = psP.tile([128, LC], F32, tag="gen", bufs=2)
                    nc.tensor.matmul(gps[:], gateWT_sb[:], xnc[:, cs],
                                     start=True, stop=True)
                    nc.scalar.activation(gate[:, cs], gps[:], AF.Sigmoid,
                                         bias=gateb_sb[:, 0:1])
                    for gl in range(2):
                        rows = slice(gl * 64, gl * 64 + 64)
                        xzp = psP.tile([128, LC], F32, tag="gen", bufs=2)
                        nc.tensor.matmul(xzp[rows, :],
                                         winTu_sb[:, gl * DI:(gl + 1) * DI],
                                         xnc[:, cs], start=True, stop=True)
                        nc.scalar.copy(u_pad[rows, 3 + c * LC: 3 + (c + 1) * LC],
                                       xzp[rows, :])
                        xzp2 = psP.tile([128, LC], F32, tag="gen", bufs=2)
                        nc.tensor.matmul(xzp2[rows, :],
                                         winTz_sb[:, gl * DI:(gl + 1) * DI],
                                         xnc[:, cs], start=True, stop=True)
                        sgz = pA.tile([128, LC], BF, tag="sgz")
                        nc.scalar.activation(sgz[rows, :], xzp2[rows, :], AF.Sigmoid)
                        nc.vector.scalar_tensor_tensor(
                            zs[rows, cs], xzp2[rows, :], 0.0, sgz[rows, :],
                            op0=ALU.add, op1=ALU.mult)

            # ---- Phase B: per (group, direction, L-half) front-end + scan ----
            LH = L // 2
            NCC = LH // LC  # 4 front-end chunks per half
            with tc.tile_pool(name="pB", bufs=2) as wp:
                for gl in range(2):
                    rows = slice(gl * 64, gl * 64 + 64)
                    for dr in range(2):
                        i4 = gl * 2 + dr
                        h_prev = [None] * NJ
                        horder = (0, 1) if dr == 0 else (1, 0)
                        for hf in horder:
                            uc_h = wp.tile([128, LH], BF, tag="uc_h", bufs=2)
                            sgd_h = wp.tile([128, LH], BF, tag="sgd_h", bufs=2)
                            bc_h = wp.tile([DS, 2 * LH], BF, tag="bc_h", bufs=2)
                            # front-end (natural order); sigmoid table set
                            for cc in range(NCC):
                                c = hf * NCC + cc
                                ccs = slice(cc * LC, (cc + 1) * LC)
                                ucp = psP.tile([128, LC], F32, tag="gen", bufs=2)
                                for k in range(DC):
                                    off = (c * LC + k) if dr == 0 else (3 + c * LC + k)
                                    nc.tensor.matmul(
                                        ucp[:],
                                        conv4T_sb[rows,
                                                  (i4 * 4 + k) * 128:
                                                  (i4 * 4 + k + 1) * 128],
                                        u_pad[rows, off:off + LC],
                                        start=(k == 0), stop=(k == DC - 1))
                                sgu = wp.tile([128, LC], BF, tag="sgu")
                                nc.scalar.activation(sgu[:], ucp[:], AF.Sigmoid,
                                                     bias=convb_sb[:, i4:i4 + 1])
                                nc.vector.scalar_tensor_tensor(
                                    uc_h[:, ccs], ucp[:], convb_sb[:, i4:i4 + 1],
                                    sgu[:], op0=ALU.add, op1=ALU.mult)
                                dtp = psP.tile([128, LC], F32, tag="gen", bufs=2)
                                nc.tensor.matmul(dtp[:],
                                                 dtWT_sb[:, i4 * 128:(i4 + 1) * 128],
                                                 uc_h[0:DI, ccs],
                                                 start=True, stop=True)
                                nc.scalar.activation(sgd_h[:, ccs], dtp[:], AF.Sigmoid,
                                                     bias=dtb_sb[:, i4:i4 + 1],
                                                     scale=-1.0)
                                bcpB = psP.tile([DS, LC], F32, tag="gen", bufs=2,
                                                name="bcpB")
                                nc.tensor.matmul(
                                    bcpB[:],
                                    xprojBCT_sb[:, i4 * 2 * DS:i4 * 2 * DS + DS],
                                    uc_h[0:DI, ccs], start=True, stop=True)
                                nc.scalar.copy(bc_h[:, cc * LC:(cc + 1) * LC],
                                               bcpB[:])
                                bcpC = psP.tile([DS, LC], F32, tag="gen", bufs=2,
                                                name="bcpC")
                                nc.tensor.matmul(
                                    bcpC[:],
                                    xprojBCT_sb[:, i4 * 2 * DS + DS:(i4 + 1) * 2 * DS],
                                    uc_h[0:DI, ccs], start=True, stop=True)
                                nc.scalar.copy(bc_h[:, LH + cc * LC:LH + (cc + 1) * LC],
                                               bcpC[:])
                            # lnexp table set from here on
                            nc.scalar.activation(sgd_h[:], sgd_h[:], AF.Ln)
                            dt_h = sgd_h
                            dtuc = wp.tile([128, LH], BF, tag="dtuc", bufs=2)
                            nc.gpsimd.tensor_tensor(dtuc[:], dt_h[:], uc_h[:],
                                                    op=ALU.mult)
                            ys = [psP.tile([128, LC], F32, tag=f"ys{q}", bufs=1,
                                           name=f"ys{q}")
                                  for q in range(NCC)]
                            for j in range(NJ):
                                dA = wp.tile([128, LH], BF, tag="dA")
                                nc.scalar.activation(
                                    dA[:], dt_h[:], AF.Exp,
                                    scale=asc_sb[:, i4 * NJ + j:i4 * NJ + j + 1])
                                dBu = wp.tile([128, LH], BF, tag="dBu")
                                for q in range(NCC):
                                    qs = slice(q * LC, (q + 1) * LC)
                                    bbB = psP.tile([128, LC], F32, tag="bbB",
                                                   bufs=1)
                                    nc.tensor.matmul(
                                        bbB[:], selBC_sb[:, j * 128:(j + 1) * 128],
                                        bc_h[:, q * LC:(q + 1) * LC],
                                        start=True, stop=True)
                                    nc.vector.tensor_tensor(dBu[:, qs], dtuc[:, qs],
                                                            bbB[:], op=ALU.mult)
                                h = hp.tile([128, LH], BF, tag="h")
                                first = (hf == horder[0])
                                hc = hp.tile([128, 1], BF, tag=f"hc{j}",
                                             name=f"hc{j}")
                                if dr == 0:
                                    init = 0.0 if first else h_prev[j][:, 0:1]
                                    nc.vector.tensor_tensor_scan(
                                        h[:], dA[:], dBu[:], init,
                                        op0=ALU.mult, op1=ALU.add)
                                    nc.scalar.copy(hc[:], h[:, LH - 1:LH])
                                else:
                                    init = 0.0 if first else h_prev[j][:, 0:1]
                                    nc.vector.tensor_tensor_scan(
                                        h[:, ::-1], dA[:, ::-1], dBu[:, ::-1], init,
                                        op0=ALU.mult, op1=ALU.add)
                                    nc.scalar.copy(hc[:], h[:, 0:1])
                                h_prev[j] = hc
                                prod = wp.tile([128, LH], BF, tag="prod")
                                for q in range(NCC):
                                    qs = slice(q * LC, (q + 1) * LC)
                                    bbC = psP.tile([128, LC], F32, tag="bbC",
                                                   bufs=1)
                                    nc.tensor.matmul(
                                        bbC[:], selBC_sb[:, j * 128:(j + 1) * 128],
                                        bc_h[:, LH + q * LC:LH + (q + 1) * LC],
                                        start=True, stop=True)
                                    nc.vector.tensor_tensor(prod[:, qs], h[:, qs],
                                                            bbC[:], op=ALU.mult)
                                    nc.tensor.matmul(
                                        ys[q][rows, :], mredM_sb[:, 0:DI],
                                        prod[:, qs],
                                        start=(j == 0), stop=(j == NJ - 1))
                            for q in range(NCC):
                                c = hf * NCC + q
                                cs = slice(c * LC, (c + 1) * LC)
                                ccs = slice(q * LC, (q + 1) * LC)
                                y1 = wp.tile([128, LC], BF, tag="y1")
                                nc.vector.scalar_tensor_tensor(
                                    y1[rows, :], uc_h[rows, ccs],
                                    dsk_sb[rows, i4:i4 + 1],
                                    ys[q][rows, :], op0=ALU.mult, op1=ALU.subtract)
                                if dr == 0:
                                    nc.vector.tensor_tensor(yfb[rows, cs],
                                                            y1[rows, :],
                                                            zs[rows, cs],
                                                            op=ALU.mult)
                                else:
                                    y2 = wp.tile([128, LC], BF, tag="y2")
                                    nc.vector.tensor_tensor(y2[rows, :], y1[rows, :],
                                                            zs[rows, cs],
                                                            op=ALU.mult)
                                    nc.gpsimd.tensor_tensor(yfb[rows, cs],
                                                             yfb[rows, cs],
                                                             y2[rows, :],
                                                             op=ALU.add)

            # ---- Phase C: Wout, exchange, blend, proj ----
            with tc.tile_pool(name="pC", bufs=2) as wpc:
                for c in range(NCH):
                    cs = slice(c * LC, (c + 1) * LC)
                    ymp = psP.tile([2 * DM, LC], F32, tag="gen", bufs=2)
                    nc.tensor.matmul(ymp[:], woutT_sb[:], yfb[:, cs],
                                     start=True, stop=True)
                    ym_sb = wpc.tile([2 * DM, LC], F32, tag="ymsb")
                    nc.scalar.copy(ym_sb[:], ymp[:])
                    nc.sync.dma_start(xm_loc[:, cs], ym_sb[:])
                nc.gpsimd.collective_compute(
                    "AllGather", ALU.bypass,
                    replica_groups=[[0, 1], [2, 3], [4, 5], [6, 7]],
                    ins=[xm_loc[:]], outs=[xm_all[:]])
                for c in range(NCH):
                    cs = slice(c * LC, (c + 1) * LC)
                    xm_t = wpc.tile([C, LC], F32, tag="xmt")
                    nc.sync.dma_start(xm_t[:], xm_all[:, cs])
                    ta = wpc.tile([128, LC], F32, tag="ta")
                    nc.vector.tensor_tensor(ta[:], xm_t[:], xs[:, cs],
                                            op=ALU.subtract)
                    tb2 = wpc.tile([128, LC], F32, tag="tb")
                    nc.vector.tensor_tensor(tb2[:], gate[:, cs], ta[:], op=ALU.mult)
                    tc2 = wpc.tile([128, LC], F32, tag="tc")
                    nc.vector.tensor_tensor(tc2[:], xs[:, cs], tb2[:], op=ALU.add)
                    op_ = psP.tile([128, LC], F32, tag="gen", bufs=2)
                    nc.tensor.matmul(op_[:], projT_sb[:], tc2[:], start=True, stop=True)
                    osb = wpc.tile([128, LC], F32, tag="osb")
                    nc.scalar.activation(osb[:], op_[:], AF.Identity,
                                         bias=projb_sb[:, 0:1])
                    nc.sync.dma_start(outp[:, cs], osb[:])
    nc.finalize()
    return nc


def _bf(a):
    import concourse.mybir as _mb
    return np.asarray(a).astype(_mb.dt.np(_mb.dt.bfloat16))


def _prep_inputs(inputs):
    """Build the 8 per-core in_maps from full inputs."""
    ii = {k: np.asarray(v, dtype=np.float32) for k, v in inputs.items()}
    x = ii["x"]

    maps_w = []  # weight dicts per group-set gs=0,1
    for gs in range(2):
        w = {}
        w9 = np.zeros((C, 9 * 128), np.float32)
        for tap in range(9):
            dy, dx = tap // 3, tap % 3
            blk = np.zeros((C, 128), np.float32)
            np.fill_diagonal(blk, ii["pos_conv_w"][:, 0, dy, dx])
            if tap == 4:
                blk[np.arange(C), np.arange(C)] += 1.0
            w9[:, tap * 128:(tap + 1) * 128] = blk
        w["w9"] = w9
        w["pe_b"] = np.ascontiguousarray(ii["pos_embed"][0].T) \
            + ii["pos_conv_b"][:, None]
        w["mred1"] = np.full((128, 1), 1.0 / 128, np.float32)
        w["onesr"] = np.ones((1, 128), np.float32)
        w["ln_g"] = np.ascontiguousarray(ii["ln_g"][:, None])
        w["ln_b"] = np.ascontiguousarray(ii["ln_b"][:, None])
        w["gateWT"] = np.ascontiguousarray(ii["gate_W"].T)
        w["gateb"] = np.ascontiguousarray(ii["gate_b"][:, None])
        w["projT"] = np.ascontiguousarray(ii["proj_W"].T)
        w["projb"] = np.ascontiguousarray(ii["proj_b"][:, None])
        w["mredM"] = _bf(np.tile(np.eye(DI, dtype=np.float32), (2, 1)))
        selBC = np.zeros((NJ, DS, 128), np.float32)
        for j in range(NJ):
            for p in range(128):
                selBC[j, 2 * j + p // 64, p] = 1.0
        w["selBC"] = _bf(selBC)
        winTu = np.zeros((2, C, DI), np.float32)
        winTz = np.zeros((2, C, DI), np.float32)
        conv4T = np.zeros((2, 2, DC, DI, 128), np.float32)
        convb = np.zeros((2, 2, 128, 1), np.float32)
        dtWT = np.zeros((2, 2, DI, 128), np.float32)
        dtb = np.zeros((2, 2, 128, 1), np.float32)
        xprojBCT = np.zeros((2, 2, DI, 2 * DS), np.float32)
        A_sc = np.zeros((2, 2, 128, NJ), np.float32)
        dsk = np.zeros((2, 2, 128, 1), np.float32)
        woutT = np.zeros((128, 2 * DM), np.float32)
        for gl in range(2):
            gg = gs * 2 + gl
            gsl = slice(gg * DM, (gg + 1) * DM)
            winTu[gl, gsl, :] = ii["m_Win"][gg, 0:DI, :].T
            winTz[gl, gsl, :] = ii["m_Win"][gg, DI:2 * DI, :].T
            woutT[gl * 64:(gl + 1) * 64, gl * DM:(gl + 1) * DM] = ii["m_Wout"][gg].T
            for dr in range(2):
                for k in range(DC):
                    wk = ii["conv_w"][gg, dr, :, k if dr == 0 else DC - 1 - k]
                    blk = np.zeros((DI, 128), np.float32)
                    blk[np.arange(DI), np.arange(DI)] = wk
                    blk[np.arange(DI), 64 + np.arange(DI)] = wk
                    conv4T[gl, dr, k] = blk
                convb[gl, dr, :, 0] = np.tile(ii["conv_b"][gg, dr], 2)
                M2 = ii["dt_W"][gg, dr] @ ii["xproj_W"][gg, dr][0:DTR, :]  # (DI, DI)
                dtWT[gl, dr] = np.concatenate([M2.T, M2.T], axis=1)  # [DI, 128]
                dtb[gl, dr, :, 0] = -np.tile(ii["dt_b"][gg, dr], 2)
                xprojBCT[gl, dr] = ii["xproj_W"][gg, dr][DTR:DTR + 2 * DS, :].T
                A = np.exp(ii["A_log"][gg, dr])  # (DI, DS); dt is negated, so +exp
                p = np.arange(128)
                for j in range(NJ):
                    A_sc[gl, dr, :, j] = A[p % 64, 2 * j + p // 64]
                dsk[gl, dr, :, 0] = np.tile(ii["Dskip"][gg, dr], 2)
        w.update(winTu=winTu, winTz=winTz, conv4T=_bf(conv4T), convb=convb,
                 dtWT=_bf(dtWT), dtb=dtb, xprojBCT=_bf(xprojBCT), A_sc=A_sc,
                 dsk=dsk, woutT=_bf(woutT))
        maps_w.append(w)

    in_maps = []
    for k in range(NCORE):
        b, gs = k // 2, k % 2
        m = dict(maps_w[gs])
        xp = np.zeros((C, 66, 66), np.float32)
        xp[:, 1:65, 1:65] = x[b]
        m["xpad"] = np.ascontiguousarray(xp.reshape(C, 66 * 66))
        in_maps.append(m)
    return in_maps


_CACHE = {}


def kernel(**inputs):
    from concourse.bass_utils import run_bass_kernel_spmd
    if "nc" not in _CACHE:
        _CACHE["nc"] = _build_nc()
    nc = _CACHE["nc"]
    in_maps = _prep_inputs(inputs)
    res = run_bass_kernel_spmd(nc, in_maps, list(range(NCORE))).results
    out = np.stack([np.asarray(res[2 * b]["outp"]).reshape(OUT, H, W)
                    for b in range(B)])
    return out.astype(np.float32)



# revision 13
# speedup vs baseline: 1.1850x; 1.1850x over previous
"""Trainium2 Bass kernel for CDMamba ModifiedSRCMLayer (self-contained), v2.

Sharding: 8 cores; core k handles batch k//2 and mamba group-pair k%2.
Group outputs are exchanged with a paired AllGather (bf16); the post-stage
(gate blend + output projection) is computed redundantly on both cores of a
pair and the host reads even cores.

v2 vs v1:
- Scan tile layout (4s x 32d): tile (g, dh, sg) holds s = sg*4 + p//32,
  d = dh*32 + p%32.  B/C/dt/dtu expansions to 128 partitions are done by
  single partition-broadcast DMAs from DRAM staging (off-engine), replacing
  ~500 PE broadcast matmuls and letting dBu/prod run as bf16 SBUF DVE TTs
  in 2x mode instead of 1x PSUM-source TTs.
- dA = exp(A_sc * dt) on ACT with per-partition scale, reading the
  DMA-broadcast dt tile.
- Phase A matmuls in fp32r (1 cyc/col instead of 2 for fp32).
- s-reduction via one [128->32] 0/1 matmul per tile accumulating over sg
  in PSUM.
- Front-end computed once per direction for both groups stacked ([2g x 64]
  rows, no 2-copy duplication).
"""
import sys
import numpy as np

for _p in ("/opt/trn_rl_repo",):
    if _p not in sys.path:
        sys.path.append(_p)

import concourse.bass as bass
import concourse.mybir as mybir
from concourse.bacc import Bacc
from concourse.tile import TileContext

# Model dims (hardcoded per the problem spec)
B, C, H, W = 4, 128, 64, 64
L = H * W                      # 4096
G, DM = 4, 32
DI, DS, DC = 64, 16, 4
DTR = 2
OUT = 128
EPS = 1e-5

NCORE = 8
LC = 512                       # front-end chunk
LH = 2048                      # half
F32 = mybir.dt.float32
F32R = mybir.dt.float32r
BF = mybir.dt.bfloat16
AF = mybir.ActivationFunctionType
ALU = mybir.AluOpType

# which (sg) tiles run dBu / prod on the Pool engine instead of DVE
POOL_DBU_SG = (1, 3)
POOL_PROD_SG = (3,)


def _build_nc():
    nc = Bacc(num_devices=NCORE)

    def inp(name, shape, dt=F32):
        return nc.dram_tensor(name, list(shape), dt, kind="ExternalInput")

    xpad = inp("xpad", (C, 66 * 66), F32R)
    pe_b = inp("pe_b", (C, L))
    w9 = inp("w9", (C, 9 * 128), F32R)
    mred1 = inp("mred1", (128, 1), F32R)
    onesr = inp("onesr", (1, 128), F32R)
    ln_g = inp("ln_g", (128, 1))
    ln_b = inp("ln_b", (128, 1))
    gateWT = inp("gateWT", (128, 128), F32R)
    gateb = inp("gateb", (128, 1))
    winTu = inp("winTu", (C, 128), F32R)
    winTz = inp("winTz", (C, 128), F32R)
    conv4T = inp("conv4T", (2, DC, 128, 128), BF)
    convb = inp("convb", (2, 128, 1))
    dtWT = inp("dtWT", (2, 128, 128), BF)
    dtb = inp("dtb", (2, 128, 1))
    xprojBCT = inp("xprojBCT", (2, 128, 64), BF)
    asc = inp("asc", (128, 32))
    dsk = inp("dsk", (2, 128, 1))
    mred64 = inp("mred64", (128, 2 * 64), BF)
    woutT = inp("woutT", (128, 64), BF)
    projT = inp("projT", (128, 128), BF)
    projb = inp("projb", (128, 1))

    dtst = nc.dram_tensor("dtst", [2, 128, L], BF)
    dtucst = nc.dram_tensor("dtucst", [2, 128, L], BF)
    bcst = nc.dram_tensor("bcst", [2, 64, L], BF)
    xm_loc = nc.dram_tensor("xm_loc", [64, L], BF)
    xm_all = nc.dram_tensor("xm_all", [C, L], BF)
    outp = nc.dram_tensor("outp", [OUT, L], F32, kind="ExternalOutput")

    with TileContext(nc) as tc:
        with (
            tc.tile_pool(name="const", bufs=1) as cp,
            tc.tile_pool(name="big", bufs=1) as bp,
            tc.tile_pool(name="hpool", bufs=2) as hp,
            tc.tile_pool(name="psP", bufs=1, space="PSUM") as psP,
        ):
            # ---- constants to SBUF ----
            def c_load(ap_dram, shape, nm, dt=F32):
                t = cp.tile(list(shape), dt, name=nm, tag=nm)
                nc.sync.dma_start(t[:], ap_dram)
                return t

            w9_sb = c_load(w9[:], (C, 9 * 128), "w9sb", F32R)
            mred1_sb = c_load(mred1[:], (128, 1), "mred1sb", F32R)
            onesr_sb = c_load(onesr[:], (1, 128), "onesrsb", F32R)
            lng_sb = c_load(ln_g[:], (128, 1), "lngsb")
            lnb_sb = c_load(ln_b[:], (128, 1), "lnbsb")
            gateWT_sb = c_load(gateWT[:], (128, 128), "gateWTsb", F32R)
            gateb_sb = c_load(gateb[:], (128, 1), "gatebsb")
            winTu_sb = c_load(winTu[:], (C, 128), "winTusb", F32R)
            winTz_sb = c_load(winTz[:], (C, 128), "winTzsb", F32R)
            conv4T_sb = cp.tile([128, 2 * DC * 128], BF)
            for dr in range(2):
                for k in range(DC):
                    nc.sync.dma_start(
                        conv4T_sb[:, (dr * DC + k) * 128:(dr * DC + k + 1) * 128],
                        conv4T[dr, k])
            convb_sb = cp.tile([128, 2], F32)
            dtb_sb = cp.tile([128, 2], F32)
            dsk_sb = cp.tile([128, 2], F32)
            dtWT_sb = cp.tile([128, 2 * 128], BF)
            xprojBCT_sb = cp.tile([128, 2 * 64], BF)
            for dr in range(2):
                nc.sync.dma_start(convb_sb[:, dr:dr + 1], convb[dr])
                nc.sync.dma_start(dtb_sb[:, dr:dr + 1], dtb[dr])
                nc.sync.dma_start(dsk_sb[:, dr:dr + 1], dsk[dr])
                nc.sync.dma_start(dtWT_sb[:, dr * 128:(dr + 1) * 128], dtWT[dr])
                nc.sync.dma_start(xprojBCT_sb[:, dr * 64:(dr + 1) * 64],
                                  xprojBCT[dr])
            asc_sb = c_load(asc[:], (128, 32), "ascsb")
            mred64_sb = cp.tile([128, 2 * 64], BF, name="mred64sb", tag="mred64sb")
            nc.sync.dma_start(mred64_sb[:], mred64[:])
            woutT_sb = cp.tile([128, 64], BF, name="woutTsb", tag="woutTsb")
            nc.sync.dma_start(woutT_sb[:], woutT[:])
            projT_sb = cp.tile([128, 128], BF, name="projTsb", tag="projTsb")
            nc.sync.dma_start(projT_sb[:], projT[:])
            projb_sb = c_load(projb[:], (128, 1), "projbsb")
            eps_sb = cp.tile([1, 1], F32)
            nc.vector.memset(eps_sb[:], EPS)

            # ---- persistent tiles ----
            xsbf = bp.tile([C, L], BF)     # post pos-embed input (bf16, phase C)
            gate = bp.tile([C, L], BF)
            u_pad = bp.tile([C, L + 6], BF)  # rows [2g x 64 u]; 3-zero halo
            zs = bp.tile([C, L], BF)       # silu(z)
            yfb = bp.tile([C, L], BF)      # y_fwd + y_bwd

            nc.vector.memset(u_pad[:, 0:3], 0.0)
            nc.vector.memset(u_pad[:, L + 3:L + 6], 0.0)

            # ---- Phase A: conv-pos-enc + pos-embed + LN, gate + xz ----
            with tc.tile_pool(name="pA", bufs=2) as pA:
                xpad_sb = pA.tile([C, 66 * 66], F32R, bufs=1)
                nc.sync.dma_start(xpad_sb[:], xpad[:])
                xpad3 = xpad_sb[:].rearrange("p (r q) -> p r q", q=66)
                xs = pA.tile([C, L], F32R, bufs=1)
                xnc = pA.tile([C, L], F32R, bufs=1)
                for c in range(8):
                    cs = slice(c * LC, (c + 1) * LC)
                    pa = psP.tile([128, 8, 64], F32, tag="gen", bufs=2)
                    for tap in range(9):
                        dy, dx = tap // 3, tap % 3
                        nc.tensor.matmul(
                            pa[:],
                            w9_sb[:, tap * 128:(tap + 1) * 128],
                            xpad3[:, c * 8 + dy:c * 8 + dy + 8, dx:dx + 64],
                            start=(tap == 0), stop=(tap == 8))
                    paf = pa[:].rearrange("p a b -> p (a b)")
                    pe_t = pA.tile([128, LC], F32, tag="pe")
                    nc.sync.dma_start(pe_t[:], pe_b[:, cs])
                    nc.vector.tensor_tensor(xs[:, cs], paf, pe_t[:], op=ALU.add)
                    nc.vector.tensor_copy(xsbf[:, cs], xs[:, cs])

                    mu = psP.tile([1, LC], F32, tag="gen", bufs=2)
                    nc.tensor.matmul(mu[:], mred1_sb[:], xs[:, cs],
                                     start=True, stop=True)
                    mu_sb = pA.tile([1, LC], F32R, tag="musb")
                    nc.scalar.copy(mu_sb[:], mu[:])
                    mub = psP.tile([128, LC], F32, tag="gen", bufs=2)
                    nc.tensor.matmul(mub[:], onesr_sb[:], mu_sb[:],
                                     start=True, stop=True)
                    xc = pA.tile([128, LC], F32R, tag="xc")
                    nc.vector.tensor_tensor(xc[:], xs[:, cs], mub[:],
                                            op=ALU.subtract)
                    xsq = pA.tile([128, LC], F32R, tag="xsq")
                    nc.scalar.square(xsq[:], xc[:])
                    var = psP.tile([1, LC], F32, tag="gen", bufs=2)
                    nc.tensor.matmul(var[:], mred1_sb[:], xsq[:],
                                     start=True, stop=True)
                    sd = pA.tile([1, LC], F32, tag="sd")
                    nc.scalar.activation(sd[:], var[:], AF.Sqrt,
                                         bias=eps_sb[:, 0:1])
                    rstd = pA.tile([1, LC], F32R, tag="rstd")
                    with nc.allow_low_precision(reason="f32r rstd for f32r matmul"):
                        nc.vector.reciprocal(rstd[:], sd[:])
                    rstdb = psP.tile([128, LC], F32, tag="gen", bufs=2)
                    nc.tensor.matmul(rstdb[:], onesr_sb[:], rstd[:],
                                     start=True, stop=True)
                    xng = pA.tile([128, LC], F32R, tag="xng")
                    nc.vector.tensor_tensor(xng[:], xc[:], rstdb[:], op=ALU.mult)
                    nc.scalar.activation(xnc[:, cs], xng[:], AF.Identity,
                                         bias=lnb_sb[:, 0:1], scale=lng_sb[:, 0:1])

                for c in range(8):
                    cs = slice(c * LC, (c + 1) * LC)
                    gps = psP.tile([128, LC], F32, tag="gen", bufs=2)
                    nc.tensor.matmul(gps[:], gateWT_sb[:], xnc[:, cs],
                                     start=True, stop=True)
                    nc.scalar.activation(gate[:, cs], gps[:], AF.Sigmoid,
                                         bias=gateb_sb[:, 0:1])
                    xzp = psP.tile([128, LC], F32, tag="gen", bufs=2)
                    nc.tensor.matmul(xzp[:], winTu_sb[:], xnc[:, cs],
                                     start=True, stop=True)
                    nc.scalar.copy(u_pad[:, 3 + c * LC: 3 + (c + 1) * LC], xzp[:])
                    xzp2 = psP.tile([128, LC], F32, tag="gen", bufs=2)
                    nc.tensor.matmul(xzp2[:], winTz_sb[:], xnc[:, cs],
                                     start=True, stop=True)
                    sgz = pA.tile([128, LC], BF, tag="sgz")
                    nc.scalar.activation(sgz[:], xzp2[:], AF.Sigmoid)
                    nc.vector.scalar_tensor_tensor(
                        zs[:, cs], xzp2[:], 0.0, sgz[:],
                        op0=ALU.add, op1=ALU.mult)

            # ---- Phase B ----
            with (
                tc.tile_pool(name="pDr", bufs=2) as pdr,
                tc.tile_pool(name="pW", bufs=2) as wp,
                tc.tile_pool(name="pBC", bufs=3) as bcp_pool,
            ):
                for dr in range(2):
                    uc2 = pdr.tile([128, L], BF, tag="uc2")
                    sgd = pdr.tile([128, L], BF, tag="sgd")
                    hc_prev = {}
                    halves = (0, 1) if dr == 0 else (1, 0)
                    for hf in halves:
                        hs = slice(hf * LH, (hf + 1) * LH)
                        # --- front-end: 4 chunks of this half ---
                        for cc in range(4):
                            c = hf * 4 + cc
                            cs = slice(c * LC, (c + 1) * LC)
                            ucp = psP.tile([128, LC], F32, tag="fe", bufs=2)
                            for k in range(DC):
                                off = (c * LC + k) if dr == 0 else (3 + c * LC + k)
                                nc.tensor.matmul(
                                    ucp[:],
                                    conv4T_sb[:, (dr * DC + k) * 128:
                                              (dr * DC + k + 1) * 128],
                                    u_pad[:, off:off + LC],
                                    start=(k == 0), stop=(k == DC - 1))
                            sgu = wp.tile([128, LC], BF, tag="sgu")
                            nc.scalar.activation(sgu[:], ucp[:], AF.Sigmoid,
                                                 bias=convb_sb[:, dr:dr + 1])
                            nc.vector.scalar_tensor_tensor(
                                uc2[:, cs], ucp[:], convb_sb[:, dr:dr + 1],
                                sgu[:], op0=ALU.add, op1=ALU.mult)
                            dtp = psP.tile([128, LC], F32, tag="fe", bufs=2)
                            nc.tensor.matmul(dtp[:],
                                             dtWT_sb[:, dr * 128:(dr + 1) * 128],
                                             uc2[:, cs], start=True, stop=True)
                            nc.scalar.activation(sgd[:, cs], dtp[:], AF.Sigmoid,
                                                 bias=dtb_sb[:, dr:dr + 1],
                                                 scale=-1.0)
                            bcps = psP.tile([128, LC], F32, tag="fe", bufs=2)
                            nc.tensor.matmul(bcps[0:64, :],
                                             xprojBCT_sb[:, dr * 64:(dr + 1) * 64],
                                             uc2[:, cs], start=True, stop=True)
                            bc_sb = wp.tile([64, LC], BF, tag="bcsb")
                            nc.scalar.copy(bc_sb[:], bcps[0:64, :])
                            nc.sync.dma_start(bcst[dr][:, cs], bc_sb[:])
                        # dt_h = ln(sigmoid) = -softplus; dtuc = dt_h * uc
                        nc.scalar.activation(sgd[:, hs], sgd[:, hs], AF.Ln)
                        dtuc = wp.tile([128, LH], BF, tag="dtuc")
                        nc.vector.tensor_tensor(dtuc[:], sgd[:, hs], uc2[:, hs],
                                                op=ALU.mult)
                        nc.sync.dma_start(dtst[dr][:, hs], sgd[:, hs])
                        nc.sync.dma_start(dtucst[dr][:, hs], dtuc[:])

                        # --- scan phase for this half ---
                        ys = [psP.tile([128, LC], F32, tag=f"ys{q}", bufs=1,
                                       name=f"ys{q}")
                              for q in range(4)]
                        first = (hf == halves[0])
                        for g in range(2):
                            for dh in range(2):
                                rows = slice(g * 64, g * 64 + 64)
                                dtbb = bcp_pool.tile([128, LH], BF, tag="dtbb")
                                nc.sync.dma_start(
                                    dtbb[:],
                                    dtst[dr][g * 64 + dh * 32:
                                             g * 64 + dh * 32 + 32, hs]
                                    .unsqueeze(0).broadcast_to((4, 32, LH)))
                                dtubb = bcp_pool.tile([128, LH], BF, tag="dtubb")
                                nc.sync.dma_start(
                                    dtubb[:],
                                    dtucst[dr][g * 64 + dh * 32:
                                               g * 64 + dh * 32 + 32, hs]
                                    .unsqueeze(0).broadcast_to((4, 32, LH)))
                                for sg in range(4):
                                    bbB = bcp_pool.tile([128, LH], BF, tag="bbB")
                                    nc.sync.dma_start(
                                        bbB[:],
                                        bcst[dr][g * 32 + sg * 4:
                                                 g * 32 + sg * 4 + 4, hs]
                                        .unsqueeze(1).broadcast_to((4, 32, LH)))
                                    bbC = bcp_pool.tile([128, LH], BF, tag="bbC")
                                    nc.sync.dma_start(
                                        bbC[:],
                                        bcst[dr][g * 32 + 16 + sg * 4:
                                                 g * 32 + 16 + sg * 4 + 4, hs]
                                        .unsqueeze(1).broadcast_to((4, 32, LH)))
                                    col = dr * 16 + g * 8 + dh * 4 + sg
                                    dA = wp.tile([128, LH], BF, tag="dA")
                                    nc.scalar.activation(
                                        dA[:], dtbb[:], AF.Exp,
                                        scale=asc_sb[:, col:col + 1])
                                    dBu = wp.tile([128, LH], BF, tag="dBu")
                                    eng = (nc.gpsimd if sg in POOL_DBU_SG
                                           else nc.vector)
                                    eng.tensor_tensor(dBu[:], dtubb[:], bbB[:],
                                                      op=ALU.mult)
                                    h = wp.tile([128, LH], BF, tag="h")
                                    ki = g * 8 + dh * 4 + sg
                                    init = 0.0 if first else hc_prev[ki][:, 0:1]
                                    hc = hp.tile([128, 1], BF, tag=f"hc{ki}",
                                                 name=f"hc{ki}")
                                    hc_prev[ki] = hc
                                    if dr == 0:
                                        nc.vector.tensor_tensor_scan(
                                            h[:], dA[:], dBu[:], init,
                                            op0=ALU.mult, op1=ALU.add)
                                        nc.scalar.copy(hc[:], h[:, LH - 1:LH])
                                    else:
                                        nc.vector.tensor_tensor_scan(
                                            h[:, ::-1], dA[:, ::-1],
                                            dBu[:, ::-1], init,
                                            op0=ALU.mult, op1=ALU.add)
                                        nc.scalar.copy(hc[:], h[:, 0:1])
                                    prod = wp.tile([128, LH], BF, tag="prod")
                                    eng2 = (nc.gpsimd if sg in POOL_PROD_SG
                                            else nc.vector)
                                    eng2.tensor_tensor(prod[:], h[:], bbC[:],
                                                       op=ALU.mult)
                                    for q in range(4):
                                        qs = slice(q * LC, (q + 1) * LC)
                                        nc.tensor.matmul(
                                            ys[q][rows, :],
                                            mred64_sb[:, dh * 64:(dh + 1) * 64],
                                            prod[:, qs],
                                            start=(dh == 0 and sg == 0),
                                            stop=(dh == 1 and sg == 3))
                        # --- tail for this half ---
                        for q in range(4):
                            c = hf * 4 + q
                            cs = slice(c * LC, (c + 1) * LC)
                            y1 = wp.tile([128, LC], BF, tag="y1")
                            nc.vector.scalar_tensor_tensor(
                                y1[:], uc2[:, cs], dsk_sb[:, dr:dr + 1],
                                ys[q][:], op0=ALU.mult, op1=ALU.subtract)
                            if dr == 0:
                                nc.vector.tensor_tensor(yfb[:, cs], y1[:],
                                                        zs[:, cs], op=ALU.mult)
                            else:
                                y2 = wp.tile([128, LC], BF, tag="y2")
                                nc.vector.tensor_tensor(y2[:], y1[:],
                                                        zs[:, cs], op=ALU.mult)
                                nc.gpsimd.tensor_tensor(yfb[:, cs], yfb[:, cs],
                                                        y2[:], op=ALU.add)

            # ---- Phase C: Wout, exchange, blend, proj ----
            with tc.tile_pool(name="pC", bufs=2) as wpc:
                for c in range(8):
                    cs = slice(c * LC, (c + 1) * LC)
                    ymp = psP.tile([128, LC], F32, tag="fe", bufs=2)
                    nc.tensor.matmul(ymp[0:64, :], woutT_sb[:], yfb[:, cs],
                                     start=True, stop=True)
                    ym_sb = wpc.tile([64, LC], BF, tag="ymsb")
                    nc.scalar.copy(ym_sb[:], ymp[0:64, :])
                    nc.sync.dma_start(xm_loc[:, cs], ym_sb[:])
                nc.gpsimd.collective_compute(
                    "AllGather", ALU.bypass,
                    replica_groups=[[0, 1], [2, 3], [4, 5], [6, 7]],
                    ins=[xm_loc[:]], outs=[xm_all[:]])
                for c in range(8):
                    cs = slice(c * LC, (c + 1) * LC)
                    xm_t = wpc.tile([C, LC], BF, tag="xmt")
                    nc.sync.dma_start(xm_t[:], xm_all[:, cs])
                    ta = wpc.tile([128, LC], BF, tag="ta")
                    nc.vector.tensor_tensor(ta[:], xm_t[:], xsbf[:, cs],
                                            op=ALU.subtract)
                    tb2 = wpc.tile([128, LC], BF, tag="tb")
                    nc.vector.tensor_tensor(tb2[:], gate[:, cs], ta[:],
                                            op=ALU.mult)
                    tc2 = wpc.tile([128, LC], BF, tag="tc")
                    nc.vector.tensor_tensor(tc2[:], xsbf[:, cs], tb2[:],
                                            op=ALU.add)
                    op_ = psP.tile([128, LC], F32, tag="fe", bufs=2)
                    nc.tensor.matmul(op_[:], projT_sb[:], tc2[:],
                                     start=True, stop=True)
                    osb = wpc.tile([128, LC], F32, tag="osb")
                    nc.scalar.activation(osb[:], op_[:], AF.Identity,
                                         bias=projb_sb[:, 0:1])
                    nc.sync.dma_start(outp[:, cs], osb[:])
    nc.finalize()
    return nc


def _bf(a):
    import concourse.mybir as _mb
    return np.asarray(a).astype(_mb.dt.np(_mb.dt.bfloat16))


def _prep_inputs(inputs):
    """Build the 8 per-core in_maps from full inputs."""
    ii = {k: np.asarray(v, dtype=np.float32) for k, v in inputs.items()}
    x = ii["x"]

    maps_w = []  # weight dicts per group-set gs=0,1
    for gs in range(2):
        w = {}
        w9 = np.zeros((C, 9 * 128), np.float32)
        for tap in range(9):
            dy, dx = tap // 3, tap % 3
            blk = np.zeros((C, 128), np.float32)
            np.fill_diagonal(blk, ii["pos_conv_w"][:, 0, dy, dx])
            if tap == 4:
                blk[np.arange(C), np.arange(C)] += 1.0
            w9[:, tap * 128:(tap + 1) * 128] = blk
        w["w9"] = w9
        w["pe_b"] = np.ascontiguousarray(ii["pos_embed"][0].T) \
            + ii["pos_conv_b"][:, None]
        w["mred1"] = np.full((128, 1), 1.0 / 128, np.float32)
        w["onesr"] = np.ones((1, 128), np.float32)
        w["ln_g"] = np.ascontiguousarray(ii["ln_g"][:, None])
        w["ln_b"] = np.ascontiguousarray(ii["ln_b"][:, None])
        w["gateWT"] = np.ascontiguousarray(ii["gate_W"].T)
        w["gateb"] = np.ascontiguousarray(ii["gate_b"][:, None])
        w["projT"] = _bf(ii["proj_W"].T)
        w["projb"] = np.ascontiguousarray(ii["proj_b"][:, None])
        mred64 = np.zeros((128, 2 * 64), np.float32)
        for dh in range(2):
            mred64[np.arange(128), dh * 64 + dh * 32 + np.arange(128) % 32] = 1.0
        w["mred64"] = _bf(mred64)

        winTu = np.zeros((C, 128), np.float32)
        winTz = np.zeros((C, 128), np.float32)
        conv4T = np.zeros((2, DC, 128, 128), np.float32)
        convb = np.zeros((2, 128, 1), np.float32)
        dtWT = np.zeros((2, 128, 128), np.float32)
        dtb = np.zeros((2, 128, 1), np.float32)
        xprojBCT = np.zeros((2, 128, 64), np.float32)
        asc = np.zeros((128, 32), np.float32)
        dsk = np.zeros((2, 128, 1), np.float32)
        woutT = np.zeros((128, 64), np.float32)
        p = np.arange(128)
        for g in range(2):
            gg = gs * 2 + g
            gsl = slice(gg * DM, (gg + 1) * DM)
            gr = slice(g * 64, (g + 1) * 64)
            winTu[gsl, g * 64:(g + 1) * 64] = ii["m_Win"][gg, 0:DI, :].T
            winTz[gsl, g * 64:(g + 1) * 64] = ii["m_Win"][gg, DI:2 * DI, :].T
            woutT[gr, g * 32:(g + 1) * 32] = ii["m_Wout"][gg].T
            for dr in range(2):
                for k in range(DC):
                    wk = ii["conv_w"][gg, dr, :, k if dr == 0 else DC - 1 - k]
                    conv4T[dr, k, g * 64 + np.arange(DI), g * 64 + np.arange(DI)] = wk
                convb[dr, gr, 0] = ii["conv_b"][gg, dr]
                M2 = ii["dt_W"][gg, dr] @ ii["xproj_W"][gg, dr][0:DTR, :]  # (DI,DI)
                dtWT[dr, gr, g * 64:(g + 1) * 64] = M2.T
                dtb[dr, gr, 0] = -ii["dt_b"][gg, dr]
                # cols g*32 + [B(16) | C(16)]
                xprojBCT[dr, gr, g * 32:g * 32 + 2 * DS] = \
                    ii["xproj_W"][gg, dr][DTR:DTR + 2 * DS, :].T
                A = np.exp(ii["A_log"][gg, dr])  # (DI, DS); dt negated -> +exp
                for dh in range(2):
                    for sg in range(4):
                        col = dr * 16 + g * 8 + dh * 4 + sg
                        asc[:, col] = A[dh * 32 + p % 32, sg * 4 + p // 32]
                dsk[dr, gr, 0] = ii["Dskip"][gg, dr]
        w.update(winTu=winTu, winTz=winTz, conv4T=_bf(conv4T), convb=convb,
                 dtWT=_bf(dtWT), dtb=dtb, xprojBCT=_bf(xprojBCT), asc=asc,
                 dsk=dsk, woutT=_bf(woutT))
        maps_w.append(w)

    in_maps = []
    for k in range(NCORE):
        b, gs = k // 2, k % 2
        m = dict(maps_w[gs])
        xp = np.zeros((C, 66, 66), np.float32)
        xp[:, 1:65, 1:65] = x[b]
        m["xpad"] = np.ascontiguousarray(xp.reshape(C, 66 * 66))
        in_maps.append(m)
    return in_maps


_CACHE = {}


def kernel(**inputs):
    from concourse.bass_utils import run_bass_kernel_spmd
    if "nc" not in _CACHE:
        _CACHE["nc"] = _build_nc()
    nc = _CACHE["nc"]
    in_maps = _prep_inputs(inputs)
    res = run_bass_kernel_spmd(nc, in_maps, list(range(NCORE))).results
    out = np.stack([np.asarray(res[2 * b]["outp"]).reshape(OUT, H, W)
                    for b in range(B)])
    return out.astype(np.float32)


# revision 18
# speedup vs baseline: 1.2102x; 1.0213x over previous
"""Trainium2 Bass kernel for CDMamba ModifiedSRCMLayer (self-contained), v2.

Sharding: 8 cores; core k handles batch k//2 and mamba group-pair k%2.
Group outputs are exchanged with a paired AllGather (bf16); the post-stage
(gate blend + output projection) is computed redundantly on both cores of a
pair and the host reads even cores.

v2 vs v1:
- Scan tile layout (4s x 32d): tile (g, dh, sg) holds s = sg*4 + p//32,
  d = dh*32 + p%32.  B/C/dt/dtu expansions to 128 partitions are done by
  single partition-broadcast DMAs from DRAM staging (off-engine), replacing
  ~500 PE broadcast matmuls and letting dBu/prod run as bf16 SBUF DVE TTs
  in 2x mode instead of 1x PSUM-source TTs.
- dA = exp(A_sc * dt) on ACT with per-partition scale, reading the
  DMA-broadcast dt tile.
- Phase A matmuls in fp32r (1 cyc/col instead of 2 for fp32).
- s-reduction via one [128->32] 0/1 matmul per tile accumulating over sg
  in PSUM.
- Front-end computed once per direction for both groups stacked ([2g x 64]
  rows, no 2-copy duplication).
"""
import sys
import numpy as np

for _p in ("/opt/trn_rl_repo",):
    if _p not in sys.path:
        sys.path.append(_p)

import concourse.bass as bass
import concourse.mybir as mybir
from concourse.bacc import Bacc
from concourse.tile import TileContext

# Model dims (hardcoded per the problem spec)
B, C, H, W = 4, 128, 64, 64
L = H * W                      # 4096
G, DM = 4, 32
DI, DS, DC = 64, 16, 4
DTR = 2
OUT = 128
EPS = 1e-5

NCORE = 8
LC = 512                       # front-end chunk
LH = 2048                      # half
F32 = mybir.dt.float32
F32R = mybir.dt.float32r
BF = mybir.dt.bfloat16
AF = mybir.ActivationFunctionType
ALU = mybir.AluOpType

# which (sg) tiles run dBu / prod on the Pool engine instead of DVE
POOL_DBU_SG = (1, 3)
POOL_PROD_SG = (3,)


def _build_nc():
    nc = Bacc(num_devices=NCORE)

    def inp(name, shape, dt=F32):
        return nc.dram_tensor(name, list(shape), dt, kind="ExternalInput")

    xpad = inp("xpad", (C, 66 * 66), F32R)
    pe_b = inp("pe_b", (C, L))
    w9 = inp("w9", (C, 9 * 128), F32R)
    mred1 = inp("mred1", (128, 1), F32R)
    onesr = inp("onesr", (1, 128), F32R)
    ln_g = inp("ln_g", (128, 1))
    ln_b = inp("ln_b", (128, 1))
    gateWT = inp("gateWT", (128, 128), F32R)
    gateb = inp("gateb", (128, 1))
    winTu = inp("winTu", (C, 128), F32R)
    winTz = inp("winTz", (C, 128), F32R)
    conv4T = inp("conv4T", (2, DC, 128, 128), BF)
    convb = inp("convb", (2, 128, 1))
    dtWT = inp("dtWT", (2, 128, 128), BF)
    dtb = inp("dtb", (2, 128, 1))
    xprojBCT = inp("xprojBCT", (2, 128, 64), BF)
    asc = inp("asc", (128, 32))
    dsk = inp("dsk", (2, 128, 1))
    mred64 = inp("mred64", (128, 2 * 64), BF)
    woutT = inp("woutT", (128, 64), BF)
    projT = inp("projT", (128, 128), BF)
    projb = inp("projb", (128, 1))

    bcst = nc.dram_tensor("bcst", [2, 64, L], BF)
    xm_loc = nc.dram_tensor("xm_loc", [64, L], BF)
    xm_all = nc.dram_tensor("xm_all", [C, L], BF)
    outp = nc.dram_tensor("outp", [OUT, L], F32, kind="ExternalOutput")

    with TileContext(nc) as tc:
        with (
            tc.tile_pool(name="const", bufs=1) as cp,
            tc.tile_pool(name="big", bufs=1) as bp,
            tc.tile_pool(name="hpool", bufs=2) as hp,
            tc.tile_pool(name="psP", bufs=1, space="PSUM") as psP,
        ):
            # ---- constants to SBUF ----
            def c_load(ap_dram, shape, nm, dt=F32):
                t = cp.tile(list(shape), dt, name=nm, tag=nm)
                nc.sync.dma_start(t[:], ap_dram)
                return t

            w9_sb = c_load(w9[:], (C, 9 * 128), "w9sb", F32R)
            mred1_sb = c_load(mred1[:], (128, 1), "mred1sb", F32R)
            onesr_sb = c_load(onesr[:], (1, 128), "onesrsb", F32R)
            lng_sb = c_load(ln_g[:], (128, 1), "lngsb")
            lnb_sb = c_load(ln_b[:], (128, 1), "lnbsb")
            gateWT_sb = c_load(gateWT[:], (128, 128), "gateWTsb", F32R)
            gateb_sb = c_load(gateb[:], (128, 1), "gatebsb")
            winTu_sb = c_load(winTu[:], (C, 128), "winTusb", F32R)
            winTz_sb = c_load(winTz[:], (C, 128), "winTzsb", F32R)
            conv4T_sb = cp.tile([128, 2 * DC * 128], BF)
            for dr in range(2):
                for k in range(DC):
                    nc.sync.dma_start(
                        conv4T_sb[:, (dr * DC + k) * 128:(dr * DC + k + 1) * 128],
                        conv4T[dr, k])
            convb_sb = cp.tile([128, 2], F32)
            dtb_sb = cp.tile([128, 2], F32)
            dsk_sb = cp.tile([128, 2], F32)
            dtWT_sb = cp.tile([128, 2 * 128], BF)
            xprojBCT_sb = cp.tile([128, 2 * 64], BF)
            for dr in range(2):
                nc.sync.dma_start(convb_sb[:, dr:dr + 1], convb[dr])
                nc.sync.dma_start(dtb_sb[:, dr:dr + 1], dtb[dr])
                nc.sync.dma_start(dsk_sb[:, dr:dr + 1], dsk[dr])
                nc.sync.dma_start(dtWT_sb[:, dr * 128:(dr + 1) * 128], dtWT[dr])
                nc.sync.dma_start(xprojBCT_sb[:, dr * 64:(dr + 1) * 64],
                                  xprojBCT[dr])
            asc_sb = c_load(asc[:], (128, 32), "ascsb")
            mred64_sb = cp.tile([128, 2 * 64], BF, name="mred64sb", tag="mred64sb")
            nc.sync.dma_start(mred64_sb[:], mred64[:])
            woutT_sb = cp.tile([128, 64], BF, name="woutTsb", tag="woutTsb")
            nc.sync.dma_start(woutT_sb[:], woutT[:])
            projT_sb = cp.tile([128, 128], BF, name="projTsb", tag="projTsb")
            nc.sync.dma_start(projT_sb[:], projT[:])
            projb_sb = c_load(projb[:], (128, 1), "projbsb")
            eps_sb = cp.tile([1, 1], F32)
            nc.vector.memset(eps_sb[:], EPS)

            # ---- persistent tiles ----
            xsbf = bp.tile([C, L], BF)     # post pos-embed input (bf16, phase C)
            gate = bp.tile([C, L], BF)
            u_pad = bp.tile([C, L + 6], BF)  # rows [2g x 64 u]; 3-zero halo
            zs = bp.tile([C, L], BF)       # silu(z)
            yfb = bp.tile([C, L], BF)      # y_fwd + y_bwd

            nc.vector.memset(u_pad[:, 0:3], 0.0)
            nc.vector.memset(u_pad[:, L + 3:L + 6], 0.0)

            # ---- Phase A: conv-pos-enc + pos-embed + LN, gate + xz ----
            with tc.tile_pool(name="pA", bufs=2) as pA:
                xpad_sb = pA.tile([C, 66 * 66], F32R, bufs=1)
                nc.sync.dma_start(xpad_sb[:], xpad[:])
                xpad3 = xpad_sb[:].rearrange("p (r q) -> p r q", q=66)
                xs = pA.tile([C, L], F32R, bufs=1)
                xnc = pA.tile([C, L], F32R, bufs=1)
                for c in range(8):
                    cs = slice(c * LC, (c + 1) * LC)
                    pa = psP.tile([128, 8, 64], F32, tag="gen", bufs=2)
                    for tap in range(9):
                        dy, dx = tap // 3, tap % 3
                        nc.tensor.matmul(
                            pa[:],
                            w9_sb[:, tap * 128:(tap + 1) * 128],
                            xpad3[:, c * 8 + dy:c * 8 + dy + 8, dx:dx + 64],
                            start=(tap == 0), stop=(tap == 8))
                    paf = pa[:].rearrange("p a b -> p (a b)")
                    pe_t = pA.tile([128, LC], F32, tag="pe")
                    nc.sync.dma_start(pe_t[:], pe_b[:, cs])
                    nc.vector.tensor_tensor(xs[:, cs], paf, pe_t[:], op=ALU.add)
                    nc.vector.tensor_copy(xsbf[:, cs], xs[:, cs])

                    mu = psP.tile([1, LC], F32, tag="gen", bufs=2)
                    nc.tensor.matmul(mu[:], mred1_sb[:], xs[:, cs],
                                     start=True, stop=True)
                    mu_sb = pA.tile([1, LC], F32R, tag="musb")
                    nc.scalar.copy(mu_sb[:], mu[:])
                    mub = psP.tile([128, LC], F32, tag="gen", bufs=2)
                    nc.tensor.matmul(mub[:], onesr_sb[:], mu_sb[:],
                                     start=True, stop=True)
                    xc = pA.tile([128, LC], F32R, tag="xc")
                    nc.vector.tensor_tensor(xc[:], xs[:, cs], mub[:],
                                            op=ALU.subtract)
                    xsq = pA.tile([128, LC], F32R, tag="xsq")
                    nc.scalar.square(xsq[:], xc[:])
                    var = psP.tile([1, LC], F32, tag="gen", bufs=2)
                    nc.tensor.matmul(var[:], mred1_sb[:], xsq[:],
                                     start=True, stop=True)
                    lv = pA.tile([1, LC], F32, tag="lv")
                    nc.scalar.activation(lv[:], var[:], AF.Ln,
                                         bias=eps_sb[:, 0:1])
                    rstd = pA.tile([1, LC], F32R, tag="rstd")
                    nc.scalar.activation(rstd[:], lv[:], AF.Exp, scale=-0.5)
                    rstdb = psP.tile([128, LC], F32, tag="gen", bufs=2)
                    nc.tensor.matmul(rstdb[:], onesr_sb[:], rstd[:],
                                     start=True, stop=True)
                    xng = pA.tile([128, LC], F32R, tag="xng")
                    nc.vector.tensor_tensor(xng[:], xc[:], rstdb[:], op=ALU.mult)
                    nc.scalar.activation(xnc[:, cs], xng[:], AF.Identity,
                                         bias=lnb_sb[:, 0:1], scale=lng_sb[:, 0:1])

                for c in range(8):
                    cs = slice(c * LC, (c + 1) * LC)
                    gps = psP.tile([128, LC], F32, tag="gen", bufs=2)
                    nc.tensor.matmul(gps[:], gateWT_sb[:], xnc[:, cs],
                                     start=True, stop=True)
                    nc.scalar.activation(gate[:, cs], gps[:], AF.Sigmoid,
                                         bias=gateb_sb[:, 0:1])
                    xzp = psP.tile([128, LC], F32, tag="gen", bufs=2)
                    nc.tensor.matmul(xzp[:], winTu_sb[:], xnc[:, cs],
                                     start=True, stop=True)
                    nc.scalar.copy(u_pad[:, 3 + c * LC: 3 + (c + 1) * LC], xzp[:])
                    xzp2 = psP.tile([128, LC], F32, tag="gen", bufs=2)
                    nc.tensor.matmul(xzp2[:], winTz_sb[:], xnc[:, cs],
                                     start=True, stop=True)
                    sgz = pA.tile([128, LC], BF, tag="sgz")
                    nc.scalar.activation(sgz[:], xzp2[:], AF.Sigmoid)
                    nc.vector.scalar_tensor_tensor(
                        zs[:, cs], xzp2[:], 0.0, sgz[:],
                        op0=ALU.add, op1=ALU.mult)

            # ---- Phase B ----
            with (
                tc.tile_pool(name="pDr", bufs=2) as pdr,
                tc.tile_pool(name="pW", bufs=2) as wp,
                tc.tile_pool(name="pBC", bufs=3) as bcp_pool,
            ):
                for dr in range(2):
                    uc2 = pdr.tile([128, L], BF, tag="uc2")
                    sgd = pdr.tile([128, L], BF, tag="sgd")
                    hc_prev = {}
                    halves = (0, 1) if dr == 0 else (1, 0)
                    for hf in halves:
                        hs = slice(hf * LH, (hf + 1) * LH)
                        # --- front-end: 4 chunks of this half ---
                        for cc in range(4):
                            c = hf * 4 + cc
                            cs = slice(c * LC, (c + 1) * LC)
                            ucp = psP.tile([128, LC], F32, tag="fe", bufs=2)
                            for k in range(DC):
                                off = (c * LC + k) if dr == 0 else (3 + c * LC + k)
                                nc.tensor.matmul(
                                    ucp[:],
                                    conv4T_sb[:, (dr * DC + k) * 128:
                                              (dr * DC + k + 1) * 128],
                                    u_pad[:, off:off + LC],
                                    start=(k == 0), stop=(k == DC - 1))
                            sgu = wp.tile([128, LC], BF, tag="sgu")
                            nc.scalar.activation(sgu[:], ucp[:], AF.Sigmoid,
                                                 bias=convb_sb[:, dr:dr + 1])
                            nc.vector.scalar_tensor_tensor(
                                uc2[:, cs], ucp[:], convb_sb[:, dr:dr + 1],
                                sgu[:], op0=ALU.add, op1=ALU.mult)
                            dtp = psP.tile([128, LC], F32, tag="fe", bufs=2)
                            nc.tensor.matmul(dtp[:],
                                             dtWT_sb[:, dr * 128:(dr + 1) * 128],
                                             uc2[:, cs], start=True, stop=True)
                            nc.scalar.activation(sgd[:, cs], dtp[:], AF.Sigmoid,
                                                 bias=dtb_sb[:, dr:dr + 1],
                                                 scale=-1.0)
                            bcps = psP.tile([128, LC], F32, tag="fe", bufs=2)
                            nc.tensor.matmul(bcps[0:64, :],
                                             xprojBCT_sb[:, dr * 64:(dr + 1) * 64],
                                             uc2[:, cs], start=True, stop=True)
                            bc_sb = wp.tile([64, LC], BF, tag="bcsb")
                            nc.scalar.copy(bc_sb[:], bcps[0:64, :])
                            nc.sync.dma_start(bcst[dr][:, cs], bc_sb[:])
                        # dt_h = ln(sigmoid) = -softplus; dtuc = dt_h * uc
                        nc.scalar.activation(sgd[:, hs], sgd[:, hs], AF.Ln)
                        dtuc = wp.tile([128, LH], BF, tag="dtuc")
                        nc.vector.tensor_tensor(dtuc[:], sgd[:, hs], uc2[:, hs],
                                                op=ALU.mult)

                        # --- scan phase for this half ---
                        ys = [psP.tile([128, LC], F32, tag=f"ys{q}", bufs=1,
                                       name=f"ys{q}")
                              for q in range(4)]
                        first = (hf == halves[0])
                        for g in range(2):
                            rows = slice(g * 64, g * 64 + 64)
                            bbBs, bbCs = [], []
                            for sg in range(4):
                                bbB = bcp_pool.tile([128, LH], BF,
                                                    tag=f"bbB{sg}", bufs=1)
                                nc.sync.dma_start(
                                    bbB[:],
                                    bcst[dr][g * 32 + sg * 4:
                                             g * 32 + sg * 4 + 4, hs]
                                    .unsqueeze(1).broadcast_to((4, 32, LH)))
                                bbBs.append(bbB)
                                bbC = bcp_pool.tile([128, LH], BF,
                                                    tag=f"bbC{sg}", bufs=1)
                                nc.gpsimd.dma_start(
                                    bbC[:],
                                    bcst[dr][g * 32 + 16 + sg * 4:
                                             g * 32 + 16 + sg * 4 + 4, hs]
                                    .unsqueeze(1).broadcast_to((4, 32, LH)))
                                bbCs.append(bbC)
                            for dh in range(2):
                                r0 = g * 64 + dh * 32
                                dtbb = bcp_pool.tile([128, LH], BF, tag="dtbb")
                                dtubb = bcp_pool.tile([128, LH], BF, tag="dtubb")
                                for rep in range(4):
                                    rp = slice(rep * 32, rep * 32 + 32)
                                    nc.scalar.dma_start(
                                        dtbb[rp, :], sgd[r0:r0 + 32, hs])
                                    nc.scalar.dma_start(
                                        dtubb[rp, :], dtuc[r0:r0 + 32, :])
                                for sg in range(4):
                                    bbB, bbC = bbBs[sg], bbCs[sg]
                                    col = dr * 16 + g * 8 + dh * 4 + sg
                                    dA = wp.tile([128, LH], BF, tag="dA")
                                    nc.scalar.activation(
                                        dA[:], dtbb[:], AF.Exp,
                                        scale=asc_sb[:, col:col + 1])
                                    dBu = wp.tile([128, LH], BF, tag="dBu")
                                    eng = (nc.gpsimd if sg in POOL_DBU_SG
                                           else nc.vector)
                                    eng.tensor_tensor(dBu[:], dtubb[:], bbB[:],
                                                      op=ALU.mult)
                                    h = wp.tile([128, LH], BF, tag="h")
                                    ki = g * 8 + dh * 4 + sg
                                    init = 0.0 if first else hc_prev[ki][:, 0:1]
                                    hc = hp.tile([128, 1], BF, tag=f"hc{ki}",
                                                 name=f"hc{ki}")
                                    hc_prev[ki] = hc
                                    if dr == 0:
                                        nc.vector.tensor_tensor_scan(
                                            h[:], dA[:], dBu[:], init,
                                            op0=ALU.mult, op1=ALU.add)
                                        nc.scalar.copy(hc[:], h[:, LH - 1:LH])
                                    else:
                                        nc.vector.tensor_tensor_scan(
                                            h[:, ::-1], dA[:, ::-1],
                                            dBu[:, ::-1], init,
                                            op0=ALU.mult, op1=ALU.add)
                                        nc.scalar.copy(hc[:], h[:, 0:1])
                                    prod = wp.tile([128, LH], BF, tag="prod")
                                    eng2 = (nc.gpsimd if sg in POOL_PROD_SG
                                            else nc.vector)
                                    eng2.tensor_tensor(prod[:], h[:], bbC[:],
                                                       op=ALU.mult)
                                    for q in range(4):
                                        qs = slice(q * LC, (q + 1) * LC)
                                        nc.tensor.matmul(
                                            ys[q][rows, :],
                                            mred64_sb[:, dh * 64:(dh + 1) * 64],
                                            prod[:, qs],
                                            start=(dh == 0 and sg == 0),
                                            stop=(dh == 1 and sg == 3))
                        # --- tail for this half ---
                        for q in range(4):
                            c = hf * 4 + q
                            cs = slice(c * LC, (c + 1) * LC)
                            y1 = wp.tile([128, LC], BF, tag="y1")
                            nc.vector.scalar_tensor_tensor(
                                y1[:], uc2[:, cs], dsk_sb[:, dr:dr + 1],
                                ys[q][:], op0=ALU.mult, op1=ALU.subtract)
                            if dr == 0:
                                nc.vector.tensor_tensor(yfb[:, cs], y1[:],
                                                        zs[:, cs], op=ALU.mult)
                            else:
                                y2 = wp.tile([128, LC], BF, tag="y2")
                                nc.vector.tensor_tensor(y2[:], y1[:],
                                                        zs[:, cs], op=ALU.mult)
                                nc.gpsimd.tensor_tensor(yfb[:, cs], yfb[:, cs],
                                                        y2[:], op=ALU.add)

            # ---- Phase C: Wout, exchange, blend, proj ----
            with tc.tile_pool(name="pC", bufs=2) as wpc:
                for c in range(8):
                    cs = slice(c * LC, (c + 1) * LC)
                    ymp = psP.tile([128, LC], F32, tag="fe", bufs=2)
                    nc.tensor.matmul(ymp[0:64, :], woutT_sb[:], yfb[:, cs],
                                     start=True, stop=True)
                    ym_sb = wpc.tile([64, LC], BF, tag="ymsb")
                    nc.scalar.copy(ym_sb[:], ymp[0:64, :])
                    nc.sync.dma_start(xm_loc[:, cs], ym_sb[:])
                nc.gpsimd.collective_compute(
                    "AllGather", ALU.bypass,
                    replica_groups=[[0, 1], [2, 3], [4, 5], [6, 7]],
                    ins=[xm_loc[:]], outs=[xm_all[:]])
                for c in range(8):
                    cs = slice(c * LC, (c + 1) * LC)
                    xm_t = wpc.tile([C, LC], BF, tag="xmt")
                    nc.sync.dma_start(xm_t[:], xm_all[:, cs])
                    ta = wpc.tile([128, LC], BF, tag="ta")
                    nc.vector.tensor_tensor(ta[:], xm_t[:], xsbf[:, cs],
                                            op=ALU.subtract)
                    tb2 = wpc.tile([128, LC], BF, tag="tb")
                    nc.vector.tensor_tensor(tb2[:], gate[:, cs], ta[:],
                                            op=ALU.mult)
                    tc2 = wpc.tile([128, LC], BF, tag="tc")
                    nc.vector.tensor_tensor(tc2[:], xsbf[:, cs], tb2[:],
                                            op=ALU.add)
                    op_ = psP.tile([128, LC], F32, tag="fe", bufs=2)
                    nc.tensor.matmul(op_[:], projT_sb[:], tc2[:],
                                     start=True, stop=True)
                    osb = wpc.tile([128, LC], F32, tag="osb")
                    nc.scalar.activation(osb[:], op_[:], AF.Identity,
                                         bias=projb_sb[:, 0:1])
                    nc.sync.dma_start(outp[:, cs], osb[:])
    nc.finalize()
    return nc


def _bf(a):
    import concourse.mybir as _mb
    return np.asarray(a).astype(_mb.dt.np(_mb.dt.bfloat16))


def _prep_inputs(inputs):
    """Build the 8 per-core in_maps from full inputs."""
    ii = {k: np.asarray(v, dtype=np.float32) for k, v in inputs.items()}
    x = ii["x"]

    maps_w = []  # weight dicts per group-set gs=0,1
    for gs in range(2):
        w = {}
        w9 = np.zeros((C, 9 * 128), np.float32)
        for tap in range(9):
            dy, dx = tap // 3, tap % 3
            blk = np.zeros((C, 128), np.float32)
            np.fill_diagonal(blk, ii["pos_conv_w"][:, 0, dy, dx])
            if tap == 4:
                blk[np.arange(C), np.arange(C)] += 1.0
            w9[:, tap * 128:(tap + 1) * 128] = blk
        w["w9"] = w9
        w["pe_b"] = np.ascontiguousarray(ii["pos_embed"][0].T) \
            + ii["pos_conv_b"][:, None]
        w["mred1"] = np.full((128, 1), 1.0 / 128, np.float32)
        w["onesr"] = np.ones((1, 128), np.float32)
        w["ln_g"] = np.ascontiguousarray(ii["ln_g"][:, None])
        w["ln_b"] = np.ascontiguousarray(ii["ln_b"][:, None])
        w["gateWT"] = np.ascontiguousarray(ii["gate_W"].T)
        w["gateb"] = np.ascontiguousarray(ii["gate_b"][:, None])
        w["projT"] = _bf(ii["proj_W"].T)
        w["projb"] = np.ascontiguousarray(ii["proj_b"][:, None])
        mred64 = np.zeros((128, 2 * 64), np.float32)
        for dh in range(2):
            mred64[np.arange(128), dh * 64 + dh * 32 + np.arange(128) % 32] = 1.0
        w["mred64"] = _bf(mred64)

        winTu = np.zeros((C, 128), np.float32)
        winTz = np.zeros((C, 128), np.float32)
        conv4T = np.zeros((2, DC, 128, 128), np.float32)
        convb = np.zeros((2, 128, 1), np.float32)
        dtWT = np.zeros((2, 128, 128), np.float32)
        dtb = np.zeros((2, 128, 1), np.float32)
        xprojBCT = np.zeros((2, 128, 64), np.float32)
        asc = np.zeros((128, 32), np.float32)
        dsk = np.zeros((2, 128, 1), np.float32)
        woutT = np.zeros((128, 64), np.float32)
        p = np.arange(128)
        for g in range(2):
            gg = gs * 2 + g
            gsl = slice(gg * DM, (gg + 1) * DM)
            gr = slice(g * 64, (g + 1) * 64)
            winTu[gsl, g * 64:(g + 1) * 64] = ii["m_Win"][gg, 0:DI, :].T
            winTz[gsl, g * 64:(g + 1) * 64] = ii["m_Win"][gg, DI:2 * DI, :].T
            woutT[gr, g * 32:(g + 1) * 32] = ii["m_Wout"][gg].T
            for dr in range(2):
                for k in range(DC):
                    wk = ii["conv_w"][gg, dr, :, k if dr == 0 else DC - 1 - k]
                    conv4T[dr, k, g * 64 + np.arange(DI), g * 64 + np.arange(DI)] = wk
                convb[dr, gr, 0] = ii["conv_b"][gg, dr]
                M2 = ii["dt_W"][gg, dr] @ ii["xproj_W"][gg, dr][0:DTR, :]  # (DI,DI)
                dtWT[dr, gr, g * 64:(g + 1) * 64] = M2.T
                dtb[dr, gr, 0] = -ii["dt_b"][gg, dr]
                # cols g*32 + [B(16) | C(16)]
                xprojBCT[dr, gr, g * 32:g * 32 + 2 * DS] = \
                    ii["xproj_W"][gg, dr][DTR:DTR + 2 * DS, :].T
                A = np.exp(ii["A_log"][gg, dr])  # (DI, DS); dt negated -> +exp
                for dh in range(2):
                    for sg in range(4):
                        col = dr * 16 + g * 8 + dh * 4 + sg
                        asc[:, col] = A[dh * 32 + p % 32, sg * 4 + p // 32]
                dsk[dr, gr, 0] = ii["Dskip"][gg, dr]
        w.update(winTu=winTu, winTz=winTz, conv4T=_bf(conv4T), convb=convb,
                 dtWT=_bf(dtWT), dtb=dtb, xprojBCT=_bf(xprojBCT), asc=asc,
                 dsk=dsk, woutT=_bf(woutT))
        maps_w.append(w)

    in_maps = []
    for k in range(NCORE):
        b, gs = k // 2, k % 2
        m = dict(maps_w[gs])
        xp = np.zeros((C, 66, 66), np.float32)
        xp[:, 1:65, 1:65] = x[b]
        m["xpad"] = np.ascontiguousarray(xp.reshape(C, 66 * 66))
        in_maps.append(m)
    return in_maps


_CACHE = {}


def kernel(**inputs):
    from concourse.bass_utils import run_bass_kernel_spmd
    if "nc" not in _CACHE:
        _CACHE["nc"] = _build_nc()
    nc = _CACHE["nc"]
    in_maps = _prep_inputs(inputs)
    res = run_bass_kernel_spmd(nc, in_maps, list(range(NCORE))).results
    out = np.stack([np.asarray(res[2 * b]["outp"]).reshape(OUT, H, W)
                    for b in range(B)])
    return out.astype(np.float32)


# revision 27
# speedup vs baseline: 1.4645x; 1.2101x over previous
"""Trainium2 Bass kernel for CDMamba ModifiedSRCMLayer (self-contained), v2.

Sharding: 8 cores; core k handles batch k//2 and mamba group-pair k%2.
Group outputs are exchanged with a paired AllGather (bf16); the post-stage
(gate blend + output projection) is computed redundantly on both cores of a
pair and the host reads even cores.

v2 vs v1:
- Scan tile layout (4s x 32d): tile (g, dh, sg) holds s = sg*4 + p//32,
  d = dh*32 + p%32.  B/C/dt/dtu expansions to 128 partitions are done by
  single partition-broadcast DMAs from DRAM staging (off-engine), replacing
  ~500 PE broadcast matmuls and letting dBu/prod run as bf16 SBUF DVE TTs
  in 2x mode instead of 1x PSUM-source TTs.
- dA = exp(A_sc * dt) on ACT with per-partition scale, reading the
  DMA-broadcast dt tile.
- Phase A matmuls in fp32r (1 cyc/col instead of 2 for fp32).
- s-reduction via one [128->32] 0/1 matmul per tile accumulating over sg
  in PSUM.
- Front-end computed once per direction for both groups stacked ([2g x 64]
  rows, no 2-copy duplication).
"""
import sys
import numpy as np

for _p in ("/opt/trn_rl_repo",):
    if _p not in sys.path:
        sys.path.append(_p)

import concourse.bass as bass
import concourse.mybir as mybir
from concourse.bacc import Bacc
from concourse.tile import TileContext

# Model dims (hardcoded per the problem spec)
B, C, H, W = 4, 128, 64, 64
L = H * W                      # 4096
G, DM = 4, 32
DI, DS, DC = 64, 16, 4
DTR = 2
OUT = 128
EPS = 1e-5

NCORE = 8
LC = 512                       # front-end chunk
LH = 2048                      # half
F32 = mybir.dt.float32
F32R = mybir.dt.float32r
BF = mybir.dt.bfloat16
AF = mybir.ActivationFunctionType
ALU = mybir.AluOpType

# which (sg) tiles run dBu / prod on the Pool engine instead of DVE
POOL_DBU_SG = (1, 3)
POOL_PROD_SG = ()


def _build_nc():
    nc = Bacc(num_devices=NCORE)

    def inp(name, shape, dt=F32):
        return nc.dram_tensor(name, list(shape), dt, kind="ExternalInput")

    xpad = inp("xpad", (C, 66 * 66), F32R)
    pe_b = inp("pe_b", (C, L))
    w9 = inp("w9", (C, 9 * 128), F32R)
    mred1 = inp("mred1", (128, 1), F32R)
    onesr = inp("onesr", (1, 128), F32R)
    ln_g = inp("ln_g", (128, 1))
    ln_b = inp("ln_b", (128, 1))
    gateWT = inp("gateWT", (128, 128), F32R)
    gateb = inp("gateb", (128, 1))
    winTu = inp("winTu", (C, 128), F32R)
    winTz = inp("winTz", (C, 128), F32R)
    conv4T = inp("conv4T", (2, DC, 128, 128), BF)
    convb = inp("convb", (2, 128, 1))
    dtWT = inp("dtWT", (2, 128, 128), BF)
    dtb = inp("dtb", (2, 128, 1))
    xprojBCT = inp("xprojBCT", (2, 128, 64), BF)
    asc = inp("asc", (128, 32))
    dsk = inp("dsk", (2, 128, 1))
    mred64 = inp("mred64", (128, 2 * 64), BF)
    exp4 = inp("exp4", (128, 4 * 128), BF)
    woutT = inp("woutT", (128, 64), BF)
    projT = inp("projT", (128, 128), BF)
    projb = inp("projb", (128, 1))

    bcst = nc.dram_tensor("bcst", [2, 2, 64, LH], BF)
    xm_loc = nc.dram_tensor("xm_loc", [64, L], BF)
    xm_all = nc.dram_tensor("xm_all", [C, L], BF)
    outp = nc.dram_tensor("outp", [OUT, L], F32, kind="ExternalOutput")

    with TileContext(nc) as tc:
        with (
            tc.tile_pool(name="const", bufs=1) as cp,
            tc.tile_pool(name="big", bufs=1) as bp,
            tc.tile_pool(name="hpool", bufs=2) as hp,
            tc.tile_pool(name="psP", bufs=1, space="PSUM") as psP,
        ):
            # ---- constants to SBUF ----
            def c_load(ap_dram, shape, nm, dt=F32):
                t = cp.tile(list(shape), dt, name=nm, tag=nm)
                nc.sync.dma_start(t[:], ap_dram)
                return t

            w9_sb = c_load(w9[:], (C, 9 * 128), "w9sb", F32R)
            mred1_sb = c_load(mred1[:], (128, 1), "mred1sb", F32R)
            onesr_sb = c_load(onesr[:], (1, 128), "onesrsb", F32R)
            lng_sb = c_load(ln_g[:], (128, 1), "lngsb")
            lnb_sb = c_load(ln_b[:], (128, 1), "lnbsb")
            gateWT_sb = c_load(gateWT[:], (128, 128), "gateWTsb", F32R)
            gateb_sb = c_load(gateb[:], (128, 1), "gatebsb")
            winTu_sb = c_load(winTu[:], (C, 128), "winTusb", F32R)
            winTz_sb = c_load(winTz[:], (C, 128), "winTzsb", F32R)
            conv4T_sb = cp.tile([128, 2 * DC * 128], BF)
            for dr in range(2):
                for k in range(DC):
                    nc.sync.dma_start(
                        conv4T_sb[:, (dr * DC + k) * 128:(dr * DC + k + 1) * 128],
                        conv4T[dr, k])
            convb_sb = cp.tile([128, 2], F32)
            dtb_sb = cp.tile([128, 2], F32)
            dsk_sb = cp.tile([128, 2], F32)
            dtWT_sb = cp.tile([128, 2 * 128], BF)
            xprojBCT_sb = cp.tile([128, 2 * 64], BF)
            for dr in range(2):
                nc.sync.dma_start(convb_sb[:, dr:dr + 1], convb[dr])
                nc.sync.dma_start(dtb_sb[:, dr:dr + 1], dtb[dr])
                nc.sync.dma_start(dsk_sb[:, dr:dr + 1], dsk[dr])
                nc.sync.dma_start(dtWT_sb[:, dr * 128:(dr + 1) * 128], dtWT[dr])
                nc.sync.dma_start(xprojBCT_sb[:, dr * 64:(dr + 1) * 64],
                                  xprojBCT[dr])
            asc_sb = c_load(asc[:], (128, 32), "ascsb")
            mred64_sb = cp.tile([128, 2 * 64], BF, name="mred64sb", tag="mred64sb")
            nc.sync.dma_start(mred64_sb[:], mred64[:])
            exp4_sb = cp.tile([128, 4 * 128], BF, name="exp4sb", tag="exp4sb")
            nc.sync.dma_start(exp4_sb[:], exp4[:])
            woutT_sb = cp.tile([128, 64], BF, name="woutTsb", tag="woutTsb")
            nc.sync.dma_start(woutT_sb[:], woutT[:])
            projT_sb = cp.tile([128, 128], BF, name="projTsb", tag="projTsb")
            nc.sync.dma_start(projT_sb[:], projT[:])
            projb_sb = c_load(projb[:], (128, 1), "projbsb")
            eps_sb = cp.tile([1, 1], F32)
            nc.vector.memset(eps_sb[:], EPS)

            # ---- persistent tiles ----
            xsbf = bp.tile([C, L], BF)     # post pos-embed input (bf16, phase C)
            gate = bp.tile([C, L], BF)
            u_pad = bp.tile([C, L + 6], BF)  # rows [2g x 64 u]; 3-zero halo
            zs = bp.tile([C, L], BF)       # silu(z)
            yfb = bp.tile([C, L], BF)      # y_fwd + y_bwd

            nc.vector.memset(u_pad[:, 0:3], 0.0)
            nc.vector.memset(u_pad[:, L + 3:L + 6], 0.0)

            # ---- Phase A: conv-pos-enc + pos-embed + LN, gate + xz ----
            with tc.tile_pool(name="pA", bufs=2) as pA:
                xpad_sb = pA.tile([C, 66 * 66], F32R, bufs=1)
                nc.sync.dma_start(xpad_sb[:], xpad[:])
                xpad3 = xpad_sb[:].rearrange("p (r q) -> p r q", q=66)
                xs = pA.tile([C, L], F32R, bufs=1)
                xnc = pA.tile([C, L], F32R, bufs=1)
                for c in range(8):
                    cs = slice(c * LC, (c + 1) * LC)
                    pa = psP.tile([128, 8, 64], F32, tag="gen", bufs=2)
                    for tap in range(9):
                        dy, dx = tap // 3, tap % 3
                        nc.tensor.matmul(
                            pa[:],
                            w9_sb[:, tap * 128:(tap + 1) * 128],
                            xpad3[:, c * 8 + dy:c * 8 + dy + 8, dx:dx + 64],
                            start=(tap == 0), stop=(tap == 8))
                    paf = pa[:].rearrange("p a b -> p (a b)")
                    pe_t = pA.tile([128, LC], F32, tag="pe")
                    nc.sync.dma_start(pe_t[:], pe_b[:, cs])
                    nc.vector.tensor_tensor(xs[:, cs], paf, pe_t[:], op=ALU.add)
                    nc.vector.tensor_copy(xsbf[:, cs], xs[:, cs])

                    mu = psP.tile([1, LC], F32, tag="gen", bufs=2)
                    nc.tensor.matmul(mu[:], mred1_sb[:], xs[:, cs],
                                     start=True, stop=True)
                    mu_sb = pA.tile([1, LC], F32R, tag="musb")
                    nc.scalar.copy(mu_sb[:], mu[:])
                    mub = psP.tile([128, LC], F32, tag="gen", bufs=2)
                    nc.tensor.matmul(mub[:], onesr_sb[:], mu_sb[:],
                                     start=True, stop=True)
                    xc = pA.tile([128, LC], F32R, tag="xc")
                    nc.vector.tensor_tensor(xc[:], xs[:, cs], mub[:],
                                            op=ALU.subtract)
                    xsq = pA.tile([128, LC], F32R, tag="xsq")
                    nc.scalar.square(xsq[:], xc[:])
                    var = psP.tile([1, LC], F32, tag="gen", bufs=2)
                    nc.tensor.matmul(var[:], mred1_sb[:], xsq[:],
                                     start=True, stop=True)
                    lv = pA.tile([1, LC], F32, tag="lv")
                    nc.scalar.activation(lv[:], var[:], AF.Ln,
                                         bias=eps_sb[:, 0:1])
                    rstd = pA.tile([1, LC], F32R, tag="rstd")
                    nc.scalar.activation(rstd[:], lv[:], AF.Exp, scale=-0.5)
                    rstdb = psP.tile([128, LC], F32, tag="gen", bufs=2)
                    nc.tensor.matmul(rstdb[:], onesr_sb[:], rstd[:],
                                     start=True, stop=True)
                    xng = pA.tile([128, LC], F32R, tag="xng")
                    nc.vector.tensor_tensor(xng[:], xc[:], rstdb[:], op=ALU.mult)
                    nc.scalar.activation(xnc[:, cs], xng[:], AF.Identity,
                                         bias=lnb_sb[:, 0:1], scale=lng_sb[:, 0:1])

                for c in range(8):
                    cs = slice(c * LC, (c + 1) * LC)
                    gps = psP.tile([128, LC], F32, tag="gen", bufs=2)
                    nc.tensor.matmul(gps[:], gateWT_sb[:], xnc[:, cs],
                                     start=True, stop=True)
                    nc.scalar.activation(gate[:, cs], gps[:], AF.Sigmoid,
                                         bias=gateb_sb[:, 0:1])
                    xzp = psP.tile([128, LC], F32, tag="gen", bufs=2)
                    nc.tensor.matmul(xzp[:], winTu_sb[:], xnc[:, cs],
                                     start=True, stop=True)
                    nc.scalar.copy(u_pad[:, 3 + c * LC: 3 + (c + 1) * LC], xzp[:])
                    xzp2 = psP.tile([128, LC], F32, tag="gen", bufs=2)
                    nc.tensor.matmul(xzp2[:], winTz_sb[:], xnc[:, cs],
                                     start=True, stop=True)
                    sgz = pA.tile([128, LC], BF, tag="sgz")
                    nc.scalar.activation(sgz[:], xzp2[:], AF.Sigmoid)
                    nc.vector.scalar_tensor_tensor(
                        zs[:, cs], xzp2[:], 0.0, sgz[:],
                        op0=ALU.add, op1=ALU.mult)

            # ---- Phase B ----
            with (
                tc.tile_pool(name="pDr", bufs=1) as pdr,
                tc.tile_pool(name="pW", bufs=2) as wp,
                tc.tile_pool(name="pBC", bufs=3) as bcp_pool,
            ):
                uc2s, sgds, dtucs = [], [], []
                # --- front-end for both directions, all chunks ---
                for dr in range(2):
                    uc2 = pdr.tile([128, L], BF, tag=f"uc2_{dr}")
                    sgd = pdr.tile([128, L], BF, tag=f"sgd_{dr}")
                    dtuc = pdr.tile([128, L], BF, tag=f"dtuc_{dr}")
                    uc2s.append(uc2); sgds.append(sgd); dtucs.append(dtuc)
                    for c in range(8):
                        cs = slice(c * LC, (c + 1) * LC)
                        ucp = psP.tile([128, LC], F32, tag="fe", bufs=2)
                        for k in range(DC):
                            off = (c * LC + k) if dr == 0 else (3 + c * LC + k)
                            nc.tensor.matmul(
                                ucp[:],
                                conv4T_sb[:, (dr * DC + k) * 128:
                                          (dr * DC + k + 1) * 128],
                                u_pad[:, off:off + LC],
                                start=(k == 0), stop=(k == DC - 1))
                        sgu = wp.tile([128, LC], BF, tag="sgu")
                        nc.scalar.activation(sgu[:], ucp[:], AF.Sigmoid,
                                             bias=convb_sb[:, dr:dr + 1])
                        nc.vector.scalar_tensor_tensor(
                            uc2[:, cs], ucp[:], convb_sb[:, dr:dr + 1],
                            sgu[:], op0=ALU.add, op1=ALU.mult)
                        dtp = psP.tile([128, LC], F32, tag="fe", bufs=2)
                        nc.tensor.matmul(dtp[:],
                                         dtWT_sb[:, dr * 128:(dr + 1) * 128],
                                         uc2[:, cs], start=True, stop=True)
                        nc.scalar.activation(sgd[:, cs], dtp[:], AF.Sigmoid,
                                             bias=dtb_sb[:, dr:dr + 1],
                                             scale=-1.0)
                        bcps = psP.tile([128, LC], F32, tag="fe", bufs=2)
                        nc.tensor.matmul(bcps[0:64, :],
                                         xprojBCT_sb[:, dr * 64:(dr + 1) * 64],
                                         uc2[:, cs], start=True, stop=True)
                        bc_sb = wp.tile([64, LC], BF, tag="bcsb")
                        nc.scalar.copy(bc_sb[:], bcps[0:64, :])
                        nc.sync.dma_start(
                            bcst[dr, c // 4][:, (c % 4) * LC:(c % 4 + 1) * LC],
                            bc_sb[:])
                    for hf in range(2):
                        hs = slice(hf * LH, (hf + 1) * LH)
                        nc.scalar.activation(sgd[:, hs], sgd[:, hs], AF.Ln)
                        nc.vector.tensor_tensor(dtuc[:, hs], sgd[:, hs],
                                                uc2[:, hs], op=ALU.mult)

                # --- scan phases ---
                for dr in range(2):
                    uc2, sgd, dtuc = uc2s[dr], sgds[dr], dtucs[dr]
                    hc_prev = {}
                    halves = (0, 1) if dr == 0 else (1, 0)
                    for hf in halves:
                        hs = slice(hf * LH, (hf + 1) * LH)
                        ys = [psP.tile([128, LC], F32, tag=f"ys{q}", bufs=1,
                                       name=f"ys{q}")
                              for q in range(4)]
                        first = (hf == halves[0])
                        for g in range(2):
                            rows = slice(g * 64, g * 64 + 64)
                            bbBs, bbCs = [], []
                            for sg in range(4):
                                bbB = bcp_pool.tile([128, LH], BF,
                                                    tag=f"bbB{sg}", bufs=1)
                                nc.sync.dma_start(
                                    bbB[:],
                                    bcst[dr, hf][g * 32 + sg * 4:
                                                 g * 32 + sg * 4 + 4, :]
                                    .unsqueeze(1).broadcast_to((4, 32, LH)))
                                bbBs.append(bbB)
                                bbC = bcp_pool.tile([128, LH], BF,
                                                    tag=f"bbC{sg}", bufs=1)
                                nc.gpsimd.dma_start(
                                    bbC[:],
                                    bcst[dr, hf][g * 32 + 16 + sg * 4:
                                                 g * 32 + 16 + sg * 4 + 4, :]
                                    .unsqueeze(1).broadcast_to((4, 32, LH)))
                                bbCs.append(bbC)
                            for dh in range(2):
                                r0 = g * 64 + dh * 32
                                gd = g * 2 + dh
                                dtubb = bcp_pool.tile([128, LH], BF,
                                                      tag="dtubb", bufs=2)
                                for rep in range(4):
                                    rp = slice(rep * 32, rep * 32 + 32)
                                    nc.sync.dma_start(
                                        dtubb[rp, :], dtuc[r0:r0 + 32, hs])
                                dAs = [wp.tile([128, LH], BF, tag=f"dA{sg}",
                                               bufs=1, name=f"dA{sg}")
                                       for sg in range(4)]
                                for q in range(4):
                                    qs = slice(q * LC, (q + 1) * LC)
                                    qh = slice(hf * LH + q * LC,
                                               hf * LH + (q + 1) * LC)
                                    dtps = psP.tile([128, LC], F32, tag="gen",
                                                    bufs=2)
                                    nc.tensor.matmul(
                                        dtps[:],
                                        exp4_sb[:, gd * 128:(gd + 1) * 128],
                                        sgd[:, qh], start=True, stop=True)
                                    for sg in range(4):
                                        col = dr * 16 + g * 8 + dh * 4 + sg
                                        nc.scalar.activation(
                                            dAs[sg][:, qs], dtps[:], AF.Exp,
                                            scale=asc_sb[:, col:col + 1])
                                for sg in range(4):
                                    bbB, bbC = bbBs[sg], bbCs[sg]
                                    ki = g * 8 + dh * 4 + sg
                                    dA = dAs[sg]
                                    dBu = wp.tile([128, LH], BF, tag="dBu")
                                    eng = (nc.gpsimd if sg in POOL_DBU_SG
                                           else nc.vector)
                                    eng.tensor_tensor(dBu[:], dtubb[:], bbB[:],
                                                      op=ALU.mult)
                                    h = wp.tile([128, LH], BF, tag="h")
                                    init = 0.0 if first else hc_prev[ki][:, 0:1]
                                    hc = hp.tile([128, 1], BF, tag=f"hc{ki}",
                                                 name=f"hc{ki}")
                                    hc_prev[ki] = hc
                                    if dr == 0:
                                        nc.vector.tensor_tensor_scan(
                                            h[:], dA[:], dBu[:], init,
                                            op0=ALU.mult, op1=ALU.add)
                                        nc.scalar.copy(hc[:], h[:, LH - 1:LH])
                                    else:
                                        nc.vector.tensor_tensor_scan(
                                            h[:, ::-1], dA[:, ::-1],
                                            dBu[:, ::-1], init,
                                            op0=ALU.mult, op1=ALU.add)
                                        nc.scalar.copy(hc[:], h[:, 0:1])
                                    prod = wp.tile([128, LH], BF, tag="prod")
                                    eng2 = (nc.gpsimd if sg in POOL_PROD_SG
                                            else nc.vector)
                                    eng2.tensor_tensor(prod[:], h[:], bbC[:],
                                                       op=ALU.mult)
                                    for q in range(4):
                                        qs = slice(q * LC, (q + 1) * LC)
                                        nc.tensor.matmul(
                                            ys[q][rows, :],
                                            mred64_sb[:, dh * 64:(dh + 1) * 64],
                                            prod[:, qs],
                                            start=(dh == 0 and sg == 0),
                                            stop=(dh == 1 and sg == 3))
                        # --- tail for this half ---
                        for q in range(4):
                            c = hf * 4 + q
                            cs = slice(c * LC, (c + 1) * LC)
                            y1 = wp.tile([128, LC], BF, tag="y1")
                            nc.vector.scalar_tensor_tensor(
                                y1[:], uc2[:, cs], dsk_sb[:, dr:dr + 1],
                                ys[q][:], op0=ALU.mult, op1=ALU.subtract)
                            if dr == 0:
                                nc.vector.tensor_tensor(yfb[:, cs], y1[:],
                                                        zs[:, cs], op=ALU.mult)
                            else:
                                y2 = wp.tile([128, LC], BF, tag="y2")
                                nc.vector.tensor_tensor(y2[:], y1[:],
                                                        zs[:, cs], op=ALU.mult)
                                nc.gpsimd.tensor_tensor(yfb[:, cs], yfb[:, cs],
                                                        y2[:], op=ALU.add)

            # ---- Phase C: Wout, exchange, blend, proj ----
            with tc.tile_pool(name="pC", bufs=2) as wpc:
                for c in range(8):
                    cs = slice(c * LC, (c + 1) * LC)
                    ymp = psP.tile([128, LC], F32, tag="fe", bufs=2)
                    nc.tensor.matmul(ymp[0:64, :], woutT_sb[:], yfb[:, cs],
                                     start=True, stop=True)
                    ym_sb = wpc.tile([64, LC], BF, tag="ymsb")
                    nc.scalar.copy(ym_sb[:], ymp[0:64, :])
                    nc.sync.dma_start(xm_loc[:, cs], ym_sb[:])
                nc.gpsimd.collective_compute(
                    "AllGather", ALU.bypass,
                    replica_groups=[[0, 1], [2, 3], [4, 5], [6, 7]],
                    ins=[xm_loc[:]], outs=[xm_all[:]])
                for c in range(8):
                    cs = slice(c * LC, (c + 1) * LC)
                    xm_t = wpc.tile([C, LC], BF, tag="xmt")
                    nc.sync.dma_start(xm_t[:], xm_all[:, cs])
                    ta = wpc.tile([128, LC], BF, tag="ta")
                    nc.vector.tensor_tensor(ta[:], xm_t[:], xsbf[:, cs],
                                            op=ALU.subtract)
                    tb2 = wpc.tile([128, LC], BF, tag="tb")
                    nc.vector.tensor_tensor(tb2[:], gate[:, cs], ta[:],
                                            op=ALU.mult)
                    tc2 = wpc.tile([128, LC], BF, tag="tc")
                    nc.vector.tensor_tensor(tc2[:], xsbf[:, cs], tb2[:],
                                            op=ALU.add)
                    op_ = psP.tile([128, LC], F32, tag="fe", bufs=2)
                    nc.tensor.matmul(op_[:], projT_sb[:], tc2[:],
                                     start=True, stop=True)
                    osb = wpc.tile([128, LC], F32, tag="osb")
                    nc.scalar.activation(osb[:], op_[:], AF.Identity,
                                         bias=projb_sb[:, 0:1])
                    nc.sync.dma_start(outp[:, cs], osb[:])
    nc.finalize()
    return nc


def _bf(a):
    import concourse.mybir as _mb
    return np.asarray(a).astype(_mb.dt.np(_mb.dt.bfloat16))


def _prep_inputs(inputs):
    """Build the 8 per-core in_maps from full inputs."""
    ii = {k: np.asarray(v, dtype=np.float32) for k, v in inputs.items()}
    x = ii["x"]

    maps_w = []  # weight dicts per group-set gs=0,1
    for gs in range(2):
        w = {}
        w9 = np.zeros((C, 9 * 128), np.float32)
        for tap in range(9):
            dy, dx = tap // 3, tap % 3
            blk = np.zeros((C, 128), np.float32)
            np.fill_diagonal(blk, ii["pos_conv_w"][:, 0, dy, dx])
            if tap == 4:
                blk[np.arange(C), np.arange(C)] += 1.0
            w9[:, tap * 128:(tap + 1) * 128] = blk
        w["w9"] = w9
        w["pe_b"] = np.ascontiguousarray(ii["pos_embed"][0].T) \
            + ii["pos_conv_b"][:, None]
        w["mred1"] = np.full((128, 1), 1.0 / 128, np.float32)
        w["onesr"] = np.ones((1, 128), np.float32)
        w["ln_g"] = np.ascontiguousarray(ii["ln_g"][:, None])
        w["ln_b"] = np.ascontiguousarray(ii["ln_b"][:, None])
        w["gateWT"] = np.ascontiguousarray(ii["gate_W"].T)
        w["gateb"] = np.ascontiguousarray(ii["gate_b"][:, None])
        w["projT"] = _bf(ii["proj_W"].T)
        w["projb"] = np.ascontiguousarray(ii["proj_b"][:, None])
        mred64 = np.zeros((128, 2 * 64), np.float32)
        for dh in range(2):
            mred64[np.arange(128), dh * 64 + dh * 32 + np.arange(128) % 32] = 1.0
        w["mred64"] = _bf(mred64)
        exp4 = np.zeros((128, 4 * 128), np.float32)
        for g in range(2):
            for dh in range(2):
                gd = g * 2 + dh
                m = np.arange(128)
                exp4[g * 64 + dh * 32 + m % 32, gd * 128 + m] = 1.0
        w["exp4"] = _bf(exp4)

        winTu = np.zeros((C, 128), np.float32)
        winTz = np.zeros((C, 128), np.float32)
        conv4T = np.zeros((2, DC, 128, 128), np.float32)
        convb = np.zeros((2, 128, 1), np.float32)
        dtWT = np.zeros((2, 128, 128), np.float32)
        dtb = np.zeros((2, 128, 1), np.float32)
        xprojBCT = np.zeros((2, 128, 64), np.float32)
        asc = np.zeros((128, 32), np.float32)
        dsk = np.zeros((2, 128, 1), np.float32)
        woutT = np.zeros((128, 64), np.float32)
        p = np.arange(128)
        for g in range(2):
            gg = gs * 2 + g
            gsl = slice(gg * DM, (gg + 1) * DM)
            gr = slice(g * 64, (g + 1) * 64)
            winTu[gsl, g * 64:(g + 1) * 64] = ii["m_Win"][gg, 0:DI, :].T
            winTz[gsl, g * 64:(g + 1) * 64] = ii["m_Win"][gg, DI:2 * DI, :].T
            woutT[gr, g * 32:(g + 1) * 32] = ii["m_Wout"][gg].T
            for dr in range(2):
                for k in range(DC):
                    wk = ii["conv_w"][gg, dr, :, k if dr == 0 else DC - 1 - k]
                    conv4T[dr, k, g * 64 + np.arange(DI), g * 64 + np.arange(DI)] = wk
                convb[dr, gr, 0] = ii["conv_b"][gg, dr]
                M2 = ii["dt_W"][gg, dr] @ ii["xproj_W"][gg, dr][0:DTR, :]  # (DI,DI)
                dtWT[dr, gr, g * 64:(g + 1) * 64] = M2.T
                dtb[dr, gr, 0] = -ii["dt_b"][gg, dr]
                # cols g*32 + [B(16) | C(16)]
                xprojBCT[dr, gr, g * 32:g * 32 + 2 * DS] = \
                    ii["xproj_W"][gg, dr][DTR:DTR + 2 * DS, :].T
                A = np.exp(ii["A_log"][gg, dr])  # (DI, DS); dt negated -> +exp
                for dh in range(2):
                    for sg in range(4):
                        col = dr * 16 + g * 8 + dh * 4 + sg
                        asc[:, col] = A[dh * 32 + p % 32, sg * 4 + p // 32]
                dsk[dr, gr, 0] = ii["Dskip"][gg, dr]
        w.update(winTu=winTu, winTz=winTz, conv4T=_bf(conv4T), convb=convb,
                 dtWT=_bf(dtWT), dtb=dtb, xprojBCT=_bf(xprojBCT), asc=asc,
                 dsk=dsk, woutT=_bf(woutT))
        maps_w.append(w)

    in_maps = []
    for k in range(NCORE):
        b, gs = k // 2, k % 2
        m = dict(maps_w[gs])
        xp = np.zeros((C, 66, 66), np.float32)
        xp[:, 1:65, 1:65] = x[b]
        m["xpad"] = np.ascontiguousarray(xp.reshape(C, 66 * 66))
        in_maps.append(m)
    return in_maps


_CACHE = {}


def kernel(**inputs):
    from concourse.bass_utils import run_bass_kernel_spmd
    if "nc" not in _CACHE:
        _CACHE["nc"] = _build_nc()
    nc = _CACHE["nc"]
    in_maps = _prep_inputs(inputs)
    res = run_bass_kernel_spmd(nc, in_maps, list(range(NCORE))).results
    out = np.stack([np.asarray(res[2 * b]["outp"]).reshape(OUT, H, W)
                    for b in range(B)])
    return out.astype(np.float32)


# revision 53
# speedup vs baseline: 1.4679x; 1.0023x over previous
"""Trainium2 Bass kernel for CDMamba ModifiedSRCMLayer (self-contained), v2.

Sharding: 8 cores; core k handles batch k//2 and mamba group-pair k%2.
Group outputs are exchanged with a paired AllGather (bf16); the post-stage
(gate blend + output projection) is computed redundantly on both cores of a
pair and the host reads even cores.

v2 vs v1:
- Scan tile layout (4s x 32d): tile (g, dh, sg) holds s = sg*4 + p//32,
  d = dh*32 + p%32.  B/C/dt/dtu expansions to 128 partitions are done by
  single partition-broadcast DMAs from DRAM staging (off-engine), replacing
  ~500 PE broadcast matmuls and letting dBu/prod run as bf16 SBUF DVE TTs
  in 2x mode instead of 1x PSUM-source TTs.
- dA = exp(A_sc * dt) on ACT with per-partition scale, reading the
  DMA-broadcast dt tile.
- Phase A matmuls in fp32r (1 cyc/col instead of 2 for fp32).
- s-reduction via one [128->32] 0/1 matmul per tile accumulating over sg
  in PSUM.
- Front-end computed once per direction for both groups stacked ([2g x 64]
  rows, no 2-copy duplication).
"""
import sys
import numpy as np

for _p in ("/opt/trn_rl_repo",):
    if _p not in sys.path:
        sys.path.append(_p)

import concourse.bass as bass
import concourse.mybir as mybir
from concourse.bacc import Bacc
from concourse.tile import TileContext

# Model dims (hardcoded per the problem spec)
B, C, H, W = 4, 128, 64, 64
L = H * W                      # 4096
G, DM = 4, 32
DI, DS, DC = 64, 16, 4
DTR = 2
OUT = 128
EPS = 1e-5

NCORE = 8
LC = 512                       # front-end chunk
LH = 2048                      # half
F32 = mybir.dt.float32
F32R = mybir.dt.float32r
BF = mybir.dt.bfloat16
AF = mybir.ActivationFunctionType
ALU = mybir.AluOpType

# which (sg) tiles run dBu / prod on the Pool engine instead of DVE
POOL_DBU_SG = (1, 3)
POOL_PROD_SG = ()


def _build_nc():
    nc = Bacc(num_devices=NCORE)

    def inp(name, shape, dt=F32):
        return nc.dram_tensor(name, list(shape), dt, kind="ExternalInput")

    xpad = inp("xpad", (C, 66 * 66), F32R)
    pe_b = inp("pe_b", (C, L))
    w9 = inp("w9", (C, 9 * 128), F32R)
    mred1 = inp("mred1", (128, 1), F32R)
    onesr = inp("onesr", (1, 128), F32R)
    ln_g = inp("ln_g", (128, 1))
    ln_b = inp("ln_b", (128, 1))
    gateWT = inp("gateWT", (128, 128), F32R)
    gateb = inp("gateb", (128, 1))
    winTu = inp("winTu", (C, 128), F32R)
    winTz = inp("winTz", (C, 128), F32R)
    conv4T = inp("conv4T", (2, DC, 128, 128), BF)
    convb = inp("convb", (2, 128, 1))
    dtWT = inp("dtWT", (2, 128, 128), BF)
    dtb = inp("dtb", (2, 128, 1))
    xprojBCT = inp("xprojBCT", (2, 128, 64), BF)
    asc = inp("asc", (128, 32))
    dsk = inp("dsk", (2, 128, 1))
    mred64 = inp("mred64", (128, 2 * 64), BF)
    dskW = inp("dskW", (2, 128, 128), BF)
    woutT = inp("woutT", (128, 64), BF)
    projT = inp("projT", (128, 128), BF)
    projb = inp("projb", (128, 1))

    bcst = nc.dram_tensor("bcst", [2, 2, 64, LH], BF)
    xg_st = nc.dram_tensor("xg_st", [2, C, L], BF)
    xm_loc = nc.dram_tensor("xm_loc", [64, L], BF)
    xm_all = nc.dram_tensor("xm_all", [C, L], BF)
    outp = nc.dram_tensor("outp", [OUT, L], F32, kind="ExternalOutput")

    with TileContext(nc) as tc:
        with (
            tc.tile_pool(name="const", bufs=1) as cp,
            tc.tile_pool(name="big", bufs=1) as bp,
            tc.tile_pool(name="hpool", bufs=2) as hp,
            tc.tile_pool(name="psP", bufs=1, space="PSUM") as psP,
        ):
            # ---- constants to SBUF ----
            def c_load(ap_dram, shape, nm, dt=F32):
                t = cp.tile(list(shape), dt, name=nm, tag=nm)
                nc.sync.dma_start(t[:], ap_dram)
                return t

            w9_sb = c_load(w9[:], (C, 9 * 128), "w9sb", F32R)
            mred1_sb = c_load(mred1[:], (128, 1), "mred1sb", F32R)
            onesr_sb = c_load(onesr[:], (1, 128), "onesrsb", F32R)
            lng_sb = c_load(ln_g[:], (128, 1), "lngsb")
            lnb_sb = c_load(ln_b[:], (128, 1), "lnbsb")
            gateWT_sb = c_load(gateWT[:], (128, 128), "gateWTsb", F32R)
            gateb_sb = c_load(gateb[:], (128, 1), "gatebsb")
            winTu_sb = c_load(winTu[:], (C, 128), "winTusb", F32R)
            winTz_sb = c_load(winTz[:], (C, 128), "winTzsb", F32R)
            conv4T_sb = cp.tile([128, 2 * DC * 128], BF)
            for dr in range(2):
                for k in range(DC):
                    nc.sync.dma_start(
                        conv4T_sb[:, (dr * DC + k) * 128:(dr * DC + k + 1) * 128],
                        conv4T[dr, k])
            convb_sb = cp.tile([128, 2], F32)
            dtb_sb = cp.tile([128, 2], F32)
            dsk_sb = cp.tile([128, 2], F32)
            dtWT_sb = cp.tile([128, 2 * 128], BF)
            xprojBCT_sb = cp.tile([128, 2 * 64], BF)
            for dr in range(2):
                nc.sync.dma_start(convb_sb[:, dr:dr + 1], convb[dr])
                nc.sync.dma_start(dtb_sb[:, dr:dr + 1], dtb[dr])
                nc.sync.dma_start(dsk_sb[:, dr:dr + 1], dsk[dr])
                nc.sync.dma_start(dtWT_sb[:, dr * 128:(dr + 1) * 128], dtWT[dr])
                nc.sync.dma_start(xprojBCT_sb[:, dr * 64:(dr + 1) * 64],
                                  xprojBCT[dr])
            asc_sb = c_load(asc[:], (128, 32), "ascsb")
            mred64_sb = cp.tile([128, 2 * 64], BF, name="mred64sb", tag="mred64sb")
            nc.sync.dma_start(mred64_sb[:], mred64[:])
            dskW_sb = cp.tile([128, 2 * 128], BF, name="dskWsb", tag="dskWsb")
            for dr in range(2):
                nc.sync.dma_start(dskW_sb[:, dr * 128:(dr + 1) * 128], dskW[dr])
            woutT_sb = cp.tile([128, 64], BF, name="woutTsb", tag="woutTsb")
            nc.sync.dma_start(woutT_sb[:], woutT[:])
            projT_sb = cp.tile([128, 128], BF, name="projTsb", tag="projTsb")
            nc.sync.dma_start(projT_sb[:], projT[:])
            projb_sb = c_load(projb[:], (128, 1), "projbsb")
            eps_sb = cp.tile([1, 1], F32)
            nc.vector.memset(eps_sb[:], EPS)

            # ---- persistent tiles ----
            zs = bp.tile([C, L], BF)       # silu(z)
            yfb = bp.tile([C, L], BF)      # y_fwd + y_bwd
            pdr_cm = tc.tile_pool(name="pDr", bufs=1)
            pdr = pdr_cm.__enter__()
            pu_cm = tc.tile_pool(name="pU", bufs=1)
            pu = pu_cm.__enter__()
            u_pad = pu.tile([C, L + 6], BF)  # rows [2g x 64 u]; 3-zero halo
            nc.vector.memset(u_pad[:, 0:3], 0.0)
            nc.vector.memset(u_pad[:, L + 3:L + 6], 0.0)

            # ---- Phase A: conv-pos-enc + pos-embed + LN, gate + xz ----
            with tc.tile_pool(name="pA", bufs=2) as pA:
                xpad_sb = pA.tile([C, 66 * 66], F32R, bufs=1)
                nc.sync.dma_start(xpad_sb[:], xpad[:])
                xpad3 = xpad_sb[:].rearrange("p (r q) -> p r q", q=66)
                xs = pA.tile([C, L], F32R, bufs=1)
                xnc = pA.tile([C, L], F32R, bufs=1)
                for c in range(8):
                    cs = slice(c * LC, (c + 1) * LC)
                    pa = psP.tile([128, 8, 64], F32, tag="gen", bufs=2)
                    for tap in range(9):
                        dy, dx = tap // 3, tap % 3
                        nc.tensor.matmul(
                            pa[:],
                            w9_sb[:, tap * 128:(tap + 1) * 128],
                            xpad3[:, c * 8 + dy:c * 8 + dy + 8, dx:dx + 64],
                            start=(tap == 0), stop=(tap == 8))
                    paf = pa[:].rearrange("p a b -> p (a b)")
                    pe_t = pA.tile([128, LC], F32, tag="pe")
                    nc.sync.dma_start(pe_t[:], pe_b[:, cs])
                    nc.vector.tensor_tensor(xs[:, cs], paf, pe_t[:], op=ALU.add)
                    xsc = pA.tile([128, LC], BF, tag="xsc")
                    nc.vector.tensor_copy(xsc[:], xs[:, cs])
                    nc.sync.dma_start(xg_st[0][:, cs], xsc[:])

                    mu = psP.tile([1, LC], F32, tag="gen", bufs=2)
                    nc.tensor.matmul(mu[:], mred1_sb[:], xs[:, cs],
                                     start=True, stop=True)
                    mu_sb = pA.tile([1, LC], F32R, tag="musb")
                    nc.scalar.copy(mu_sb[:], mu[:])
                    mub = psP.tile([128, LC], F32, tag="gen", bufs=2)
                    nc.tensor.matmul(mub[:], onesr_sb[:], mu_sb[:],
                                     start=True, stop=True)
                    xc = pA.tile([128, LC], F32R, tag="xc")
                    nc.vector.tensor_tensor(xc[:], xs[:, cs], mub[:],
                                            op=ALU.subtract)
                    xsq = pA.tile([128, LC], F32R, tag="xsq")
                    nc.scalar.square(xsq[:], xc[:])
                    var = psP.tile([1, LC], F32, tag="gen", bufs=2)
                    nc.tensor.matmul(var[:], mred1_sb[:], xsq[:],
                                     start=True, stop=True)
                    lv = pA.tile([1, LC], F32, tag="lv")
                    nc.scalar.activation(lv[:], var[:], AF.Ln,
                                         bias=eps_sb[:, 0:1])
                    rstd = pA.tile([1, LC], F32R, tag="rstd")
                    nc.scalar.activation(rstd[:], lv[:], AF.Exp, scale=-0.5)
                    rstdb = psP.tile([128, LC], F32, tag="gen", bufs=2)
                    nc.tensor.matmul(rstdb[:], onesr_sb[:], rstd[:],
                                     start=True, stop=True)
                    xng = pA.tile([128, LC], F32R, tag="xng")
                    nc.vector.tensor_tensor(xng[:], xc[:], rstdb[:], op=ALU.mult)
                    nc.scalar.activation(xnc[:, cs], xng[:], AF.Identity,
                                         bias=lnb_sb[:, 0:1], scale=lng_sb[:, 0:1])

                for c in range(8):
                    cs = slice(c * LC, (c + 1) * LC)
                    gps = psP.tile([128, LC], F32, tag="gen", bufs=2)
                    nc.tensor.matmul(gps[:], gateWT_sb[:], xnc[:, cs],
                                     start=True, stop=True)
                    gate_c = pA.tile([128, LC], BF, tag="gatec")
                    nc.scalar.activation(gate_c[:], gps[:], AF.Sigmoid,
                                         bias=gateb_sb[:, 0:1])
                    nc.sync.dma_start(xg_st[1][:, cs], gate_c[:])
                    xzp = psP.tile([128, LC], F32, tag="gen", bufs=2)
                    nc.tensor.matmul(xzp[:], winTu_sb[:], xnc[:, cs],
                                     start=True, stop=True)
                    nc.scalar.copy(u_pad[:, 3 + c * LC: 3 + (c + 1) * LC], xzp[:])
                    xzp2 = psP.tile([128, LC], F32, tag="gen", bufs=2)
                    nc.tensor.matmul(xzp2[:], winTz_sb[:], xnc[:, cs],
                                     start=True, stop=True)
                    sgz = pA.tile([128, LC], BF, tag="sgz")
                    nc.scalar.activation(sgz[:], xzp2[:], AF.Sigmoid)
                    nc.vector.scalar_tensor_tensor(
                        zs[:, cs], xzp2[:], 0.0, sgz[:],
                        op0=ALU.add, op1=ALU.mult)

            # ---- Phase B front-end for both directions, all chunks ----
            if True:
                uc2s, sgds, dtucs = [], [], []
                for dr in range(2):
                    uc2 = pdr.tile([128, L], BF, tag=f"uc2_{dr}")
                    sgd = pdr.tile([128, L], BF, tag=f"sgd_{dr}")
                    dtuc = pdr.tile([128, L], BF, tag=f"dtuc_{dr}")
                    uc2s.append(uc2); sgds.append(sgd); dtucs.append(dtuc)
                    for c in range(8):
                        cs = slice(c * LC, (c + 1) * LC)
                        ucp = psP.tile([128, LC], F32, tag="fe", bufs=2)
                        for k in range(DC):
                            off = (c * LC + k) if dr == 0 else (3 + c * LC + k)
                            nc.tensor.matmul(
                                ucp[:],
                                conv4T_sb[:, (dr * DC + k) * 128:
                                          (dr * DC + k + 1) * 128],
                                u_pad[:, off:off + LC],
                                start=(k == 0), stop=(k == DC - 1))
                        nc.scalar.activation(uc2[:, cs], ucp[:], AF.Silu,
                                             bias=convb_sb[:, dr:dr + 1])
                    for c in range(8):
                        cs = slice(c * LC, (c + 1) * LC)
                        dtp = psP.tile([128, LC], F32, tag="fe", bufs=2)
                        nc.tensor.matmul(dtp[:],
                                         dtWT_sb[:, dr * 128:(dr + 1) * 128],
                                         uc2[:, cs], start=True, stop=True)
                        nc.scalar.activation(sgd[:, cs], dtp[:], AF.Sigmoid,
                                             bias=dtb_sb[:, dr:dr + 1],
                                             scale=-1.0)
                        bcps = psP.tile([128, LC], F32, tag="fe", bufs=2)
                        nc.tensor.matmul(bcps[0:64, :],
                                         xprojBCT_sb[:, dr * 64:(dr + 1) * 64],
                                         uc2[:, cs], start=True, stop=True)
                        bc_sb = pu.tile([64, LC], BF, tag="bcsb", bufs=2)
                        nc.scalar.copy(bc_sb[:], bcps[0:64, :])
                        nc.sync.dma_start(
                            bcst[dr, c // 4][:, (c % 4) * LC:(c % 4 + 1) * LC],
                            bc_sb[:])
                    for hf in range(2):
                        hs = slice(hf * LH, (hf + 1) * LH)
                        nc.scalar.activation(sgd[:, hs], sgd[:, hs], AF.Ln)
                        nc.vector.tensor_tensor(dtuc[:, hs], sgd[:, hs],
                                                uc2[:, hs], op=ALU.mult)
            pu_cm.__exit__(None, None, None)

            # ---- Phase B scan phases ----
            with (
                tc.tile_pool(name="pW", bufs=2) as wp,
                tc.tile_pool(name="pBC", bufs=3) as bcp_pool,
            ):
                for dr in range(2):
                    uc2, sgd, dtuc = uc2s[dr], sgds[dr], dtucs[dr]
                    hc_prev = {}
                    halves = (0, 1) if dr == 0 else (1, 0)
                    for hf in halves:
                        hs = slice(hf * LH, (hf + 1) * LH)
                        ys = [psP.tile([128, LC], F32, tag=f"ys{q}", bufs=1,
                                       name=f"ys{q}")
                              for q in range(4)]
                        for q in range(4):
                            cs = slice((hf * 4 + q) * LC, (hf * 4 + q + 1) * LC)
                            nc.tensor.matmul(
                                ys[q][:], dskW_sb[:, dr * 128:(dr + 1) * 128],
                                uc2[:, cs], start=True, stop=False,
                                skip_group_check=True)
                        first = (hf == halves[0])
                        for g in range(2):
                            rows = slice(g * 64, g * 64 + 64)
                            bbBs, bbCs = [], []
                            for sg in range(4):
                                bbB = bcp_pool.tile([128, LH], BF,
                                                    tag=f"bbB{sg}", bufs=2)
                                nc.sync.dma_start(
                                    bbB[:],
                                    bcst[dr, hf][g * 32 + sg * 4:
                                                 g * 32 + sg * 4 + 4, :]
                                    .unsqueeze(1).broadcast_to((4, 32, LH)))
                                bbBs.append(bbB)
                                bbC = bcp_pool.tile([128, LH], BF,
                                                    tag=f"bbC{sg}", bufs=2)
                                nc.sync.dma_start(
                                    bbC[:],
                                    bcst[dr, hf][g * 32 + 16 + sg * 4:
                                                 g * 32 + 16 + sg * 4 + 4, :]
                                    .unsqueeze(1).broadcast_to((4, 32, LH)))
                                bbCs.append(bbC)
                            for dh in range(2):
                                r0 = g * 64 + dh * 32
                                dtubb = bcp_pool.tile([128, LH], BF,
                                                      tag="dtubb", bufs=2)
                                dtbb = bcp_pool.tile([128, LH], BF,
                                                     tag="dtbb", bufs=2)
                                for rep in range(4):
                                    rp = slice(rep * 32, rep * 32 + 32)
                                    nc.sync.dma_start(
                                        dtubb[rp, :], dtuc[r0:r0 + 32, hs])
                                    nc.scalar.dma_start(
                                        dtbb[rp, :], sgd[r0:r0 + 32, hs])
                                for sg in range(4):
                                    bbB, bbC = bbBs[sg], bbCs[sg]
                                    ki = g * 8 + dh * 4 + sg
                                    col = dr * 16 + g * 8 + dh * 4 + sg
                                    dA = wp.tile([128, LH], BF, tag="dA",
                                                 bufs=3)
                                    nc.scalar.activation(
                                        dA[:], dtbb[:], AF.Exp,
                                        scale=asc_sb[:, col:col + 1])
                                    dBu = wp.tile([128, LH], BF, tag="dBu")
                                    eng = (nc.gpsimd if sg in POOL_DBU_SG
                                           else nc.vector)
                                    eng.tensor_tensor(dBu[:], dtubb[:], bbB[:],
                                                      op=ALU.mult)
                                    h = wp.tile([128, LH], BF, tag="h")
                                    init = 0.0 if first else hc_prev[ki][:, 0:1]
                                    hc = hp.tile([128, 1], BF, tag=f"hc{ki}",
                                                 name=f"hc{ki}")
                                    hc_prev[ki] = hc
                                    if dr == 0:
                                        nc.vector.tensor_tensor_scan(
                                            h[:], dA[:], dBu[:], init,
                                            op0=ALU.mult, op1=ALU.add)
                                        nc.vector.tensor_copy(hc[:],
                                                              h[:, LH - 1:LH])
                                    else:
                                        nc.vector.tensor_tensor_scan(
                                            h[:, ::-1], dA[:, ::-1],
                                            dBu[:, ::-1], init,
                                            op0=ALU.mult, op1=ALU.add)
                                        nc.vector.tensor_copy(hc[:], h[:, 0:1])
                                    prod = wp.tile([128, LH], BF, tag="prod")
                                    eng2 = (nc.gpsimd if sg in POOL_PROD_SG
                                            else nc.vector)
                                    eng2.tensor_tensor(prod[:], h[:], bbC[:],
                                                       op=ALU.mult)
                                    for q in range(4):
                                        qs = slice(q * LC, (q + 1) * LC)
                                        nc.tensor.matmul(
                                            ys[q][rows, :],
                                            mred64_sb[:, dh * 64:(dh + 1) * 64],
                                            prod[:, qs],
                                            start=False,
                                            stop=(dh == 1 and sg == 3),
                                            skip_group_check=True)
                        # --- tail for this half ---
                        for q in range(4):
                            c = hf * 4 + q
                            cs = slice(c * LC, (c + 1) * LC)
                            if dr == 0:
                                nc.vector.tensor_tensor(yfb[:, cs], ys[q][:],
                                                        zs[:, cs], op=ALU.mult)
                            else:
                                y2 = wp.tile([128, LC], BF, tag="y2")
                                nc.vector.tensor_tensor(y2[:], ys[q][:],
                                                        zs[:, cs], op=ALU.mult)
                                nc.gpsimd.tensor_tensor(yfb[:, cs], yfb[:, cs],
                                                        y2[:], op=ALU.add)

            pdr_cm.__exit__(None, None, None)

            # ---- Phase C: Wout, exchange, blend, proj ----
            with tc.tile_pool(name="pC", bufs=2) as wpc:
                for c in range(8):
                    cs = slice(c * LC, (c + 1) * LC)
                    ymp = psP.tile([128, LC], F32, tag="fe", bufs=2)
                    nc.tensor.matmul(ymp[0:64, :], woutT_sb[:], yfb[:, cs],
                                     start=True, stop=True)
                    ym_sb = wpc.tile([64, LC], BF, tag="ymsb")
                    nc.scalar.copy(ym_sb[:], ymp[0:64, :])
                    nc.sync.dma_start(xm_loc[:, cs], ym_sb[:])
                nc.gpsimd.collective_compute(
                    "AllGather", ALU.bypass,
                    replica_groups=[[0, 1], [2, 3], [4, 5], [6, 7]],
                    ins=[xm_loc[:]], outs=[xm_all[:]])
                for c in range(8):
                    cs = slice(c * LC, (c + 1) * LC)
                    xm_t = wpc.tile([C, LC], BF, tag="xmt")
                    nc.sync.dma_start(xm_t[:], xm_all[:, cs])
                    xs_t = wpc.tile([C, LC], BF, tag="xst")
                    nc.sync.dma_start(xs_t[:], xg_st[0][:, cs])
                    gt_t = wpc.tile([C, LC], BF, tag="gtt")
                    nc.scalar.dma_start(gt_t[:], xg_st[1][:, cs])
                    ta = wpc.tile([128, LC], BF, tag="ta")
                    nc.vector.tensor_tensor(ta[:], xm_t[:], xs_t[:],
                                            op=ALU.subtract)
                    tb2 = wpc.tile([128, LC], BF, tag="tb")
                    nc.vector.tensor_tensor(tb2[:], gt_t[:], ta[:],
                                            op=ALU.mult)
                    tc2 = wpc.tile([128, LC], BF, tag="tc")
                    nc.vector.tensor_tensor(tc2[:], xs_t[:], tb2[:],
                                            op=ALU.add)
                    op_ = psP.tile([128, LC], F32, tag="fe", bufs=2)
                    nc.tensor.matmul(op_[:], projT_sb[:], tc2[:],
                                     start=True, stop=True)
                    osb = wpc.tile([128, LC], F32, tag="osb")
                    nc.scalar.activation(osb[:], op_[:], AF.Identity,
                                         bias=projb_sb[:, 0:1])
                    nc.sync.dma_start(outp[:, cs], osb[:])
    nc.finalize()
    return nc


def _bf(a):
    import concourse.mybir as _mb
    return np.asarray(a).astype(_mb.dt.np(_mb.dt.bfloat16))


def _prep_inputs(inputs):
    """Build the 8 per-core in_maps from full inputs."""
    ii = {k: np.asarray(v, dtype=np.float32) for k, v in inputs.items()}
    x = ii["x"]

    maps_w = []  # weight dicts per group-set gs=0,1
    for gs in range(2):
        w = {}
        w9 = np.zeros((C, 9 * 128), np.float32)
        for tap in range(9):
            dy, dx = tap // 3, tap % 3
            blk = np.zeros((C, 128), np.float32)
            np.fill_diagonal(blk, ii["pos_conv_w"][:, 0, dy, dx])
            if tap == 4:
                blk[np.arange(C), np.arange(C)] += 1.0
            w9[:, tap * 128:(tap + 1) * 128] = blk
        w["w9"] = w9
        w["pe_b"] = np.ascontiguousarray(ii["pos_embed"][0].T) \
            + ii["pos_conv_b"][:, None]
        w["mred1"] = np.full((128, 1), 1.0 / 128, np.float32)
        w["onesr"] = np.ones((1, 128), np.float32)
        w["ln_g"] = np.ascontiguousarray(ii["ln_g"][:, None])
        w["ln_b"] = np.ascontiguousarray(ii["ln_b"][:, None])
        w["gateWT"] = np.ascontiguousarray(ii["gate_W"].T)
        w["gateb"] = np.ascontiguousarray(ii["gate_b"][:, None])
        w["projT"] = _bf(ii["proj_W"].T)
        w["projb"] = np.ascontiguousarray(ii["proj_b"][:, None])
        mred64 = np.zeros((128, 2 * 64), np.float32)
        for dh in range(2):
            mred64[np.arange(128), dh * 64 + dh * 32 + np.arange(128) % 32] = -1.0
        w["mred64"] = _bf(mred64)

        dskWm = np.zeros((2, 128, 128), np.float32)
        winTu = np.zeros((C, 128), np.float32)
        winTz = np.zeros((C, 128), np.float32)
        conv4T = np.zeros((2, DC, 128, 128), np.float32)
        convb = np.zeros((2, 128, 1), np.float32)
        dtWT = np.zeros((2, 128, 128), np.float32)
        dtb = np.zeros((2, 128, 1), np.float32)
        xprojBCT = np.zeros((2, 128, 64), np.float32)
        asc = np.zeros((128, 32), np.float32)
        dsk = np.zeros((2, 128, 1), np.float32)
        woutT = np.zeros((128, 64), np.float32)
        p = np.arange(128)
        for g in range(2):
            gg = gs * 2 + g
            gsl = slice(gg * DM, (gg + 1) * DM)
            gr = slice(g * 64, (g + 1) * 64)
            winTu[gsl, g * 64:(g + 1) * 64] = ii["m_Win"][gg, 0:DI, :].T
            winTz[gsl, g * 64:(g + 1) * 64] = ii["m_Win"][gg, DI:2 * DI, :].T
            woutT[gr, g * 32:(g + 1) * 32] = ii["m_Wout"][gg].T
            for dr in range(2):
                for k in range(DC):
                    wk = ii["conv_w"][gg, dr, :, k if dr == 0 else DC - 1 - k]
                    conv4T[dr, k, g * 64 + np.arange(DI), g * 64 + np.arange(DI)] = wk
                convb[dr, gr, 0] = ii["conv_b"][gg, dr]
                M2 = ii["dt_W"][gg, dr] @ ii["xproj_W"][gg, dr][0:DTR, :]  # (DI,DI)
                dtWT[dr, gr, g * 64:(g + 1) * 64] = M2.T
                dtb[dr, gr, 0] = -ii["dt_b"][gg, dr]
                # cols g*32 + [B(16) | C(16)]
                xprojBCT[dr, gr, g * 32:g * 32 + 2 * DS] = \
                    ii["xproj_W"][gg, dr][DTR:DTR + 2 * DS, :].T
                A = np.exp(ii["A_log"][gg, dr])  # (DI, DS); dt negated -> +exp
                for dh in range(2):
                    for sg in range(4):
                        col = dr * 16 + g * 8 + dh * 4 + sg
                        asc[:, col] = A[dh * 32 + p % 32, sg * 4 + p // 32]
                dsk[dr, gr, 0] = ii["Dskip"][gg, dr]
                dskWm[dr, g * 64 + np.arange(DI), g * 64 + np.arange(DI)] = \
                    ii["Dskip"][gg, dr]
        w["dskW"] = _bf(dskWm)
        w.update(winTu=winTu, winTz=winTz, conv4T=_bf(conv4T), convb=convb,
                 dtWT=_bf(dtWT), dtb=dtb, xprojBCT=_bf(xprojBCT), asc=asc,
                 dsk=dsk, woutT=_bf(woutT))
        maps_w.append(w)

    in_maps = []
    for k in range(NCORE):
        b, gs = k // 2, k % 2
        m = dict(maps_w[gs])
        xp = np.zeros((C, 66, 66), np.float32)
        xp[:, 1:65, 1:65] = x[b]
        m["xpad"] = np.ascontiguousarray(xp.reshape(C, 66 * 66))
        in_maps.append(m)
    return in_maps


_CACHE = {}


def kernel(**inputs):
    from concourse.bass_utils import run_bass_kernel_spmd
    if "nc" not in _CACHE:
        _CACHE["nc"] = _build_nc()
    nc = _CACHE["nc"]
    in_maps = _prep_inputs(inputs)
    res = run_bass_kernel_spmd(nc, in_maps, list(range(NCORE))).results
    out = np.stack([np.asarray(res[2 * b]["outp"]).reshape(OUT, H, W)
                    for b in range(B)])
    return out.astype(np.float32)


# revision 54
# speedup vs baseline: 1.6901x; 1.1513x over previous
"""Trainium2 Bass kernel for CDMamba ModifiedSRCMLayer (self-contained), v2.

Sharding: 8 cores; core k handles batch k//2 and mamba group-pair k%2.
Group outputs are exchanged with a paired AllGather (bf16); the post-stage
(gate blend + output projection) is computed redundantly on both cores of a
pair and the host reads even cores.

v2 vs v1:
- Scan tile layout (4s x 32d): tile (g, dh, sg) holds s = sg*4 + p//32,
  d = dh*32 + p%32.  B/C/dt/dtu expansions to 128 partitions are done by
  single partition-broadcast DMAs from DRAM staging (off-engine), replacing
  ~500 PE broadcast matmuls and letting dBu/prod run as bf16 SBUF DVE TTs
  in 2x mode instead of 1x PSUM-source TTs.
- dA = exp(A_sc * dt) on ACT with per-partition scale, reading the
  DMA-broadcast dt tile.
- Phase A matmuls in fp32r (1 cyc/col instead of 2 for fp32).
- s-reduction via one [128->32] 0/1 matmul per tile accumulating over sg
  in PSUM.
- Front-end computed once per direction for both groups stacked ([2g x 64]
  rows, no 2-copy duplication).
"""
import sys
import numpy as np

for _p in ("/opt/trn_rl_repo",):
    if _p not in sys.path:
        sys.path.append(_p)

import concourse.bass as bass
import concourse.mybir as mybir
from concourse.bacc import Bacc
from concourse.tile import TileContext

# Model dims (hardcoded per the problem spec)
B, C, H, W = 4, 128, 64, 64
L = H * W                      # 4096
G, DM = 4, 32
DI, DS, DC = 64, 16, 4
DTR = 2
OUT = 128
EPS = 1e-5

NCORE = 8
LC = 512                       # front-end chunk
LH = 2048                      # half
F32 = mybir.dt.float32
F32R = mybir.dt.float32r
BF = mybir.dt.bfloat16
AF = mybir.ActivationFunctionType
ALU = mybir.AluOpType

# which (sg) tiles run dBu / prod on the Pool engine instead of DVE
POOL_DBU_SG = ()
POOL_PROD_SG = ()


def _build_nc():
    nc = Bacc(num_devices=NCORE)

    def inp(name, shape, dt=F32):
        return nc.dram_tensor(name, list(shape), dt, kind="ExternalInput")

    xpad = inp("xpad", (C, 66 * 66), F32R)
    pe_b = inp("pe_b", (C, L))
    w9 = inp("w9", (C, 9 * 128), F32R)
    mred1 = inp("mred1", (128, 1), F32R)
    onesr = inp("onesr", (1, 128), F32R)
    ln_g = inp("ln_g", (128, 1))
    ln_b = inp("ln_b", (128, 1))
    gateWT = inp("gateWT", (128, 128), F32R)
    gateb = inp("gateb", (128, 1))
    winTu = inp("winTu", (C, 128), F32R)
    winTz = inp("winTz", (C, 128), F32R)
    conv4T = inp("conv4T", (2, DC, 128, 128), BF)
    convb = inp("convb", (2, 128, 1))
    dtWT = inp("dtWT", (2, 128, 128), BF)
    dtb = inp("dtb", (2, 128, 1))
    xprojBCT = inp("xprojBCT", (2, 128, 64), BF)
    asc = inp("asc", (128, 32))
    dsk = inp("dsk", (2, 128, 1))
    mred64 = inp("mred64", (128, 2 * 64), BF)
    dskW = inp("dskW", (2, 128, 128), BF)
    woutT = inp("woutT", (128, 64), BF)
    projT = inp("projT", (128, 128), BF)
    projb = inp("projb", (128, 1))

    bcst = nc.dram_tensor("bcst", [2, 2, 64, LH], BF)
    xg_st = nc.dram_tensor("xg_st", [2, C, L], BF)
    xm_loc = nc.dram_tensor("xm_loc", [64, L], BF)
    xm_all = nc.dram_tensor("xm_all", [C, L], BF)
    outp = nc.dram_tensor("outp", [OUT, L], F32, kind="ExternalOutput")

    with TileContext(nc) as tc:
        with (
            tc.tile_pool(name="const", bufs=1) as cp,
            tc.tile_pool(name="big", bufs=1) as bp,
            tc.tile_pool(name="hpool", bufs=2) as hp,
            tc.tile_pool(name="psP", bufs=1, space="PSUM") as psP,
        ):
            # ---- constants to SBUF ----
            def c_load(ap_dram, shape, nm, dt=F32):
                t = cp.tile(list(shape), dt, name=nm, tag=nm)
                nc.sync.dma_start(t[:], ap_dram)
                return t

            w9_sb = c_load(w9[:], (C, 9 * 128), "w9sb", F32R)
            mred1_sb = c_load(mred1[:], (128, 1), "mred1sb", F32R)
            onesr_sb = c_load(onesr[:], (1, 128), "onesrsb", F32R)
            lng_sb = c_load(ln_g[:], (128, 1), "lngsb")
            lnb_sb = c_load(ln_b[:], (128, 1), "lnbsb")
            gateWT_sb = c_load(gateWT[:], (128, 128), "gateWTsb", F32R)
            gateb_sb = c_load(gateb[:], (128, 1), "gatebsb")
            winTu_sb = c_load(winTu[:], (C, 128), "winTusb", F32R)
            winTz_sb = c_load(winTz[:], (C, 128), "winTzsb", F32R)
            conv4T_sb = cp.tile([128, 2 * DC * 128], BF)
            for dr in range(2):
                for k in range(DC):
                    nc.sync.dma_start(
                        conv4T_sb[:, (dr * DC + k) * 128:(dr * DC + k + 1) * 128],
                        conv4T[dr, k])
            convb_sb = cp.tile([128, 2], F32)
            dtb_sb = cp.tile([128, 2], F32)
            dsk_sb = cp.tile([128, 2], F32)
            dtWT_sb = cp.tile([128, 2 * 128], BF)
            xprojBCT_sb = cp.tile([128, 2 * 64], BF)
            for dr in range(2):
                nc.sync.dma_start(convb_sb[:, dr:dr + 1], convb[dr])
                nc.sync.dma_start(dtb_sb[:, dr:dr + 1], dtb[dr])
                nc.sync.dma_start(dsk_sb[:, dr:dr + 1], dsk[dr])
                nc.sync.dma_start(dtWT_sb[:, dr * 128:(dr + 1) * 128], dtWT[dr])
                nc.sync.dma_start(xprojBCT_sb[:, dr * 64:(dr + 1) * 64],
                                  xprojBCT[dr])
            asc_sb = c_load(asc[:], (128, 32), "ascsb")
            mred64_sb = cp.tile([128, 2 * 64], BF, name="mred64sb", tag="mred64sb")
            nc.sync.dma_start(mred64_sb[:], mred64[:])
            dskW_sb = cp.tile([128, 2 * 128], BF, name="dskWsb", tag="dskWsb")
            for dr in range(2):
                nc.sync.dma_start(dskW_sb[:, dr * 128:(dr + 1) * 128], dskW[dr])
            woutT_sb = cp.tile([128, 64], BF, name="woutTsb", tag="woutTsb")
            nc.sync.dma_start(woutT_sb[:], woutT[:])
            projT_sb = cp.tile([128, 128], BF, name="projTsb", tag="projTsb")
            nc.sync.dma_start(projT_sb[:], projT[:])
            projb_sb = c_load(projb[:], (128, 1), "projbsb")
            eps_sb = cp.tile([1, 1], F32)
            nc.vector.memset(eps_sb[:], EPS)

            # ---- persistent tiles ----
            zs = bp.tile([C, L], BF)       # silu(z)
            yfb = bp.tile([C, L], BF)      # y_fwd + y_bwd
            pdr_cm = tc.tile_pool(name="pDr", bufs=1)
            pdr = pdr_cm.__enter__()
            pu_cm = tc.tile_pool(name="pU", bufs=1)
            pu = pu_cm.__enter__()
            u_pad = pu.tile([C, L + 6], BF)  # rows [2g x 64 u]; 3-zero halo
            nc.vector.memset(u_pad[:, 0:3], 0.0)
            nc.vector.memset(u_pad[:, L + 3:L + 6], 0.0)

            # ---- Phase A: conv-pos-enc + pos-embed + LN, gate + xz ----
            with tc.tile_pool(name="pA", bufs=2) as pA:
                xpad_sb = pA.tile([C, 66 * 66], F32R, bufs=1)
                nc.sync.dma_start(xpad_sb[:], xpad[:])
                xpad3 = xpad_sb[:].rearrange("p (r q) -> p r q", q=66)
                xs = pA.tile([C, L], F32R, bufs=1)
                xnc = pA.tile([C, L], F32R, bufs=1)
                for c in range(8):
                    cs = slice(c * LC, (c + 1) * LC)
                    pa = psP.tile([128, 8, 64], F32, tag="gen", bufs=2)
                    for tap in range(9):
                        dy, dx = tap // 3, tap % 3
                        nc.tensor.matmul(
                            pa[:],
                            w9_sb[:, tap * 128:(tap + 1) * 128],
                            xpad3[:, c * 8 + dy:c * 8 + dy + 8, dx:dx + 64],
                            start=(tap == 0), stop=(tap == 8))
                    paf = pa[:].rearrange("p a b -> p (a b)")
                    pe_t = pA.tile([128, LC], F32, tag="pe")
                    nc.sync.dma_start(pe_t[:], pe_b[:, cs])
                    nc.vector.tensor_tensor(xs[:, cs], paf, pe_t[:], op=ALU.add)
                    xsc = pA.tile([128, LC], BF, tag="xsc")
                    nc.vector.tensor_copy(xsc[:], xs[:, cs])
                    nc.sync.dma_start(xg_st[0][:, cs], xsc[:])

                    mu = psP.tile([1, LC], F32, tag="ys0", bufs=1)
                    nc.tensor.matmul(mu[:], mred1_sb[:], xs[:, cs],
                                     start=True, stop=True)
                    mu_sb = pA.tile([1, LC], F32R, tag="musb")
                    nc.scalar.copy(mu_sb[:], mu[:])
                    mub = psP.tile([128, LC], F32, tag="ys1", bufs=1)
                    nc.tensor.matmul(mub[:], onesr_sb[:], mu_sb[:],
                                     start=True, stop=True)
                    xc = pA.tile([128, LC], F32R, tag="xc")
                    nc.vector.tensor_tensor(xc[:], xs[:, cs], mub[:],
                                            op=ALU.subtract)
                    xsq = pA.tile([128, LC], F32R, tag="xsq")
                    nc.scalar.square(xsq[:], xc[:])
                    var = psP.tile([1, LC], F32, tag="ys2", bufs=1)
                    nc.tensor.matmul(var[:], mred1_sb[:], xsq[:],
                                     start=True, stop=True)
                    lv = pA.tile([1, LC], F32, tag="lv")
                    nc.scalar.activation(lv[:], var[:], AF.Ln,
                                         bias=eps_sb[:, 0:1])
                    rstd = pA.tile([1, LC], F32R, tag="rstd")
                    nc.scalar.activation(rstd[:], lv[:], AF.Exp, scale=-0.5)
                    rstdb = psP.tile([128, LC], F32, tag="ys3", bufs=1)
                    nc.tensor.matmul(rstdb[:], onesr_sb[:], rstd[:],
                                     start=True, stop=True)
                    xng = pA.tile([128, LC], F32R, tag="xng")
                    nc.vector.tensor_tensor(xng[:], xc[:], rstdb[:], op=ALU.mult)
                    nc.scalar.activation(xnc[:, cs], xng[:], AF.Identity,
                                         bias=lnb_sb[:, 0:1], scale=lng_sb[:, 0:1])

                for c in range(8):
                    cs = slice(c * LC, (c + 1) * LC)
                    gps = psP.tile([128, LC], F32, tag="fe", bufs=2)
                    nc.tensor.matmul(gps[:], gateWT_sb[:], xnc[:, cs],
                                     start=True, stop=True)
                    gate_c = pA.tile([128, LC], BF, tag="gatec")
                    nc.scalar.activation(gate_c[:], gps[:], AF.Sigmoid,
                                         bias=gateb_sb[:, 0:1])
                    nc.sync.dma_start(xg_st[1][:, cs], gate_c[:])
                    xzp = psP.tile([128, LC], F32, tag="fe", bufs=2)
                    nc.tensor.matmul(xzp[:], winTu_sb[:], xnc[:, cs],
                                     start=True, stop=True)
                    nc.scalar.copy(u_pad[:, 3 + c * LC: 3 + (c + 1) * LC], xzp[:])
                    xzp2 = psP.tile([128, LC], F32, tag="fe", bufs=2)
                    nc.tensor.matmul(xzp2[:], winTz_sb[:], xnc[:, cs],
                                     start=True, stop=True)
                    sgz = pA.tile([128, LC], BF, tag="sgz")
                    nc.scalar.activation(sgz[:], xzp2[:], AF.Sigmoid)
                    nc.vector.scalar_tensor_tensor(
                        zs[:, cs], xzp2[:], 0.0, sgz[:],
                        op0=ALU.add, op1=ALU.mult)

            # ---- Phase B front-end for both directions, all chunks ----
            if True:
                uc2s, sgds, dtucs = [], [], []
                for dr in range(2):
                    uc2 = pdr.tile([128, L], BF, tag=f"uc2_{dr}")
                    sgd = pdr.tile([128, L], BF, tag=f"sgd_{dr}")
                    dtuc = pdr.tile([128, L], BF, tag=f"dtuc_{dr}")
                    uc2s.append(uc2); sgds.append(sgd); dtucs.append(dtuc)
                    for c in range(8):
                        cs = slice(c * LC, (c + 1) * LC)
                        ucp = psP.tile([128, LC], F32, tag="fe", bufs=2)
                        for k in range(DC):
                            off = (c * LC + k) if dr == 0 else (3 + c * LC + k)
                            nc.tensor.matmul(
                                ucp[:],
                                conv4T_sb[:, (dr * DC + k) * 128:
                                          (dr * DC + k + 1) * 128],
                                u_pad[:, off:off + LC],
                                start=(k == 0), stop=(k == DC - 1))
                        nc.scalar.activation(uc2[:, cs], ucp[:], AF.Silu,
                                             bias=convb_sb[:, dr:dr + 1])
                    for c in range(8):
                        cs = slice(c * LC, (c + 1) * LC)
                        dtp = psP.tile([128, LC], F32, tag="fe", bufs=2)
                        nc.tensor.matmul(dtp[:],
                                         dtWT_sb[:, dr * 128:(dr + 1) * 128],
                                         uc2[:, cs], start=True, stop=True)
                        nc.scalar.activation(sgd[:, cs], dtp[:], AF.Sigmoid,
                                             bias=dtb_sb[:, dr:dr + 1],
                                             scale=-1.0)
                        bcps = psP.tile([128, LC], F32, tag="fe", bufs=2)
                        nc.tensor.matmul(bcps[0:64, :],
                                         xprojBCT_sb[:, dr * 64:(dr + 1) * 64],
                                         uc2[:, cs], start=True, stop=True)
                        bc_sb = pu.tile([64, LC], BF, tag="bcsb", bufs=2)
                        nc.scalar.copy(bc_sb[:], bcps[0:64, :])
                        nc.sync.dma_start(
                            bcst[dr, c // 4][:, (c % 4) * LC:(c % 4 + 1) * LC],
                            bc_sb[:])
                    for hf in range(2):
                        hs = slice(hf * LH, (hf + 1) * LH)
                        nc.scalar.activation(sgd[:, hs], sgd[:, hs], AF.Ln)
                        nc.vector.tensor_tensor(dtuc[:, hs], sgd[:, hs],
                                                uc2[:, hs], op=ALU.mult)
            pu_cm.__exit__(None, None, None)

            # ---- Phase B scan phases ----
            with (
                tc.tile_pool(name="pW", bufs=2) as wp,
                tc.tile_pool(name="pBC", bufs=3) as bcp_pool,
            ):
                for dr in range(2):
                    uc2, sgd, dtuc = uc2s[dr], sgds[dr], dtucs[dr]
                    hc_prev = {}
                    halves = (0, 1) if dr == 0 else (1, 0)
                    for hf in halves:
                        hs = slice(hf * LH, (hf + 1) * LH)
                        ys = [psP.tile([128, LC], F32, tag=f"ys{q}", bufs=1,
                                       name=f"ys{q}")
                              for q in range(4)]
                        for q in range(4):
                            cs = slice((hf * 4 + q) * LC, (hf * 4 + q + 1) * LC)
                            nc.tensor.matmul(
                                ys[q][:], dskW_sb[:, dr * 128:(dr + 1) * 128],
                                uc2[:, cs], start=True, stop=False,
                                skip_group_check=True)
                        first = (hf == halves[0])
                        for g in range(2):
                            rows = slice(g * 64, g * 64 + 64)
                            bbBs, bbCs = [], []
                            for sg in range(4):
                                bbB = bcp_pool.tile([128, LH], BF,
                                                    tag=f"bbB{sg}", bufs=2)
                                nc.sync.dma_start(
                                    bbB[:],
                                    bcst[dr, hf][g * 32 + sg * 4:
                                                 g * 32 + sg * 4 + 4, :]
                                    .unsqueeze(1).broadcast_to((4, 32, LH)))
                                bbBs.append(bbB)
                                bbC = bcp_pool.tile([128, LH], BF,
                                                    tag=f"bbC{sg}", bufs=2)
                                nc.sync.dma_start(
                                    bbC[:],
                                    bcst[dr, hf][g * 32 + 16 + sg * 4:
                                                 g * 32 + 16 + sg * 4 + 4, :]
                                    .unsqueeze(1).broadcast_to((4, 32, LH)))
                                bbCs.append(bbC)
                            for dh in range(2):
                                r0 = g * 64 + dh * 32
                                dtubb = bcp_pool.tile([128, LH], BF,
                                                      tag="dtubb", bufs=2)
                                dtbb = bcp_pool.tile([128, LH], BF,
                                                     tag="dtbb", bufs=2)
                                for rep in range(4):
                                    rp = slice(rep * 32, rep * 32 + 32)
                                    nc.sync.dma_start(
                                        dtubb[rp, :], dtuc[r0:r0 + 32, hs])
                                    nc.scalar.dma_start(
                                        dtbb[rp, :], sgd[r0:r0 + 32, hs])
                                for sg in range(4):
                                    bbB, bbC = bbBs[sg], bbCs[sg]
                                    ki = g * 8 + dh * 4 + sg
                                    col = dr * 16 + g * 8 + dh * 4 + sg
                                    dA = wp.tile([128, LH], BF, tag="dA",
                                                 bufs=3)
                                    nc.scalar.activation(
                                        dA[:], dtbb[:], AF.Exp,
                                        scale=asc_sb[:, col:col + 1])
                                    dBu = wp.tile([128, LH], BF, tag="dBu")
                                    eng = (nc.gpsimd if sg in POOL_DBU_SG
                                           else nc.vector)
                                    eng.tensor_tensor(dBu[:], dtubb[:], bbB[:],
                                                      op=ALU.mult)
                                    h = wp.tile([128, LH], BF, tag="h")
                                    init = 0.0 if first else hc_prev[ki][:, 0:1]
                                    hc = hp.tile([128, 1], BF, tag=f"hc{ki}",
                                                 name=f"hc{ki}")
                                    hc_prev[ki] = hc
                                    if dr == 0:
                                        nc.vector.tensor_tensor_scan(
                                            h[:], dA[:], dBu[:], init,
                                            op0=ALU.mult, op1=ALU.add)
                                        nc.vector.tensor_copy(hc[:],
                                                              h[:, LH - 1:LH])
                                    else:
                                        nc.vector.tensor_tensor_scan(
                                            h[:, ::-1], dA[:, ::-1],
                                            dBu[:, ::-1], init,
                                            op0=ALU.mult, op1=ALU.add)
                                        nc.vector.tensor_copy(hc[:], h[:, 0:1])
                                    prod = wp.tile([128, LH], BF, tag="prod")
                                    eng2 = (nc.gpsimd if sg in POOL_PROD_SG
                                            else nc.vector)
                                    eng2.tensor_tensor(prod[:], h[:], bbC[:],
                                                       op=ALU.mult)
                                    for q in range(4):
                                        qs = slice(q * LC, (q + 1) * LC)
                                        nc.tensor.matmul(
                                            ys[q][rows, :],
                                            mred64_sb[:, dh * 64:(dh + 1) * 64],
                                            prod[:, qs],
                                            start=False,
                                            stop=(dh == 1 and sg == 3),
                                            skip_group_check=True)
                        # --- tail for this half ---
                        for q in range(4):
                            c = hf * 4 + q
                            cs = slice(c * LC, (c + 1) * LC)
                            if dr == 0:
                                nc.vector.tensor_tensor(yfb[:, cs], ys[q][:],
                                                        zs[:, cs], op=ALU.mult)
                            else:
                                y2 = wp.tile([128, LC], BF, tag="y2")
                                nc.vector.tensor_tensor(y2[:], ys[q][:],
                                                        zs[:, cs], op=ALU.mult)
                                nc.gpsimd.tensor_tensor(yfb[:, cs], yfb[:, cs],
                                                        y2[:], op=ALU.add)

            pdr_cm.__exit__(None, None, None)

            # ---- Phase C: Wout, exchange, blend, proj ----
            with tc.tile_pool(name="pC", bufs=2) as wpc:
                for c in range(8):
                    cs = slice(c * LC, (c + 1) * LC)
                    ymp = psP.tile([128, LC], F32, tag="fe", bufs=2)
                    nc.tensor.matmul(ymp[0:64, :], woutT_sb[:], yfb[:, cs],
                                     start=True, stop=True)
                    ym_sb = wpc.tile([64, LC], BF, tag="ymsb")
                    nc.scalar.copy(ym_sb[:], ymp[0:64, :])
                    nc.sync.dma_start(xm_loc[:, cs], ym_sb[:])
                nc.gpsimd.collective_compute(
                    "AllGather", ALU.bypass,
                    replica_groups=[[0, 1], [2, 3], [4, 5], [6, 7]],
                    ins=[xm_loc[:]], outs=[xm_all[:]])
                for c in range(8):
                    cs = slice(c * LC, (c + 1) * LC)
                    xm_t = wpc.tile([C, LC], BF, tag="xmt")
                    nc.sync.dma_start(xm_t[:], xm_all[:, cs])
                    xs_t = wpc.tile([C, LC], BF, tag="xst")
                    nc.sync.dma_start(xs_t[:], xg_st[0][:, cs])
                    gt_t = wpc.tile([C, LC], BF, tag="gtt")
                    nc.scalar.dma_start(gt_t[:], xg_st[1][:, cs])
                    ta = wpc.tile([128, LC], BF, tag="ta")
                    nc.vector.tensor_tensor(ta[:], xm_t[:], xs_t[:],
                                            op=ALU.subtract)
                    tb2 = wpc.tile([128, LC], BF, tag="tb")
                    nc.vector.tensor_tensor(tb2[:], gt_t[:], ta[:],
                                            op=ALU.mult)
                    tc2 = wpc.tile([128, LC], BF, tag="tc")
                    nc.vector.tensor_tensor(tc2[:], xs_t[:], tb2[:],
                                            op=ALU.add)
                    op_ = psP.tile([128, LC], F32, tag="fe", bufs=2)
                    nc.tensor.matmul(op_[:], projT_sb[:], tc2[:],
                                     start=True, stop=True)
                    osb = wpc.tile([128, LC], F32, tag="osb")
                    nc.scalar.activation(osb[:], op_[:], AF.Identity,
                                         bias=projb_sb[:, 0:1])
                    nc.sync.dma_start(outp[:, cs], osb[:])
    nc.finalize()
    return nc


def _bf(a):
    import concourse.mybir as _mb
    return np.asarray(a).astype(_mb.dt.np(_mb.dt.bfloat16))


def _prep_inputs(inputs):
    """Build the 8 per-core in_maps from full inputs."""
    ii = {k: np.asarray(v, dtype=np.float32) for k, v in inputs.items()}
    x = ii["x"]

    maps_w = []  # weight dicts per group-set gs=0,1
    for gs in range(2):
        w = {}
        w9 = np.zeros((C, 9 * 128), np.float32)
        for tap in range(9):
            dy, dx = tap // 3, tap % 3
            blk = np.zeros((C, 128), np.float32)
            np.fill_diagonal(blk, ii["pos_conv_w"][:, 0, dy, dx])
            if tap == 4:
                blk[np.arange(C), np.arange(C)] += 1.0
            w9[:, tap * 128:(tap + 1) * 128] = blk
        w["w9"] = w9
        w["pe_b"] = np.ascontiguousarray(ii["pos_embed"][0].T) \
            + ii["pos_conv_b"][:, None]
        w["mred1"] = np.full((128, 1), 1.0 / 128, np.float32)
        w["onesr"] = np.ones((1, 128), np.float32)
        w["ln_g"] = np.ascontiguousarray(ii["ln_g"][:, None])
        w["ln_b"] = np.ascontiguousarray(ii["ln_b"][:, None])
        w["gateWT"] = np.ascontiguousarray(ii["gate_W"].T)
        w["gateb"] = np.ascontiguousarray(ii["gate_b"][:, None])
        w["projT"] = _bf(ii["proj_W"].T)
        w["projb"] = np.ascontiguousarray(ii["proj_b"][:, None])
        mred64 = np.zeros((128, 2 * 64), np.float32)
        for dh in range(2):
            mred64[np.arange(128), dh * 64 + dh * 32 + np.arange(128) % 32] = -1.0
        w["mred64"] = _bf(mred64)

        dskWm = np.zeros((2, 128, 128), np.float32)
        winTu = np.zeros((C, 128), np.float32)
        winTz = np.zeros((C, 128), np.float32)
        conv4T = np.zeros((2, DC, 128, 128), np.float32)
        convb = np.zeros((2, 128, 1), np.float32)
        dtWT = np.zeros((2, 128, 128), np.float32)
        dtb = np.zeros((2, 128, 1), np.float32)
        xprojBCT = np.zeros((2, 128, 64), np.float32)
        asc = np.zeros((128, 32), np.float32)
        dsk = np.zeros((2, 128, 1), np.float32)
        woutT = np.zeros((128, 64), np.float32)
        p = np.arange(128)
        for g in range(2):
            gg = gs * 2 + g
            gsl = slice(gg * DM, (gg + 1) * DM)
            gr = slice(g * 64, (g + 1) * 64)
            winTu[gsl, g * 64:(g + 1) * 64] = ii["m_Win"][gg, 0:DI, :].T
            winTz[gsl, g * 64:(g + 1) * 64] = ii["m_Win"][gg, DI:2 * DI, :].T
            woutT[gr, g * 32:(g + 1) * 32] = ii["m_Wout"][gg].T
            for dr in range(2):
                for k in range(DC):
                    wk = ii["conv_w"][gg, dr, :, k if dr == 0 else DC - 1 - k]
                    conv4T[dr, k, g * 64 + np.arange(DI), g * 64 + np.arange(DI)] = wk
                convb[dr, gr, 0] = ii["conv_b"][gg, dr]
                M2 = ii["dt_W"][gg, dr] @ ii["xproj_W"][gg, dr][0:DTR, :]  # (DI,DI)
                dtWT[dr, gr, g * 64:(g + 1) * 64] = M2.T
                dtb[dr, gr, 0] = -ii["dt_b"][gg, dr]
                # cols g*32 + [B(16) | C(16)]
                xprojBCT[dr, gr, g * 32:g * 32 + 2 * DS] = \
                    ii["xproj_W"][gg, dr][DTR:DTR + 2 * DS, :].T
                A = np.exp(ii["A_log"][gg, dr])  # (DI, DS); dt negated -> +exp
                for dh in range(2):
                    for sg in range(4):
                        col = dr * 16 + g * 8 + dh * 4 + sg
                        asc[:, col] = A[dh * 32 + p % 32, sg * 4 + p // 32]
                dsk[dr, gr, 0] = ii["Dskip"][gg, dr]
                dskWm[dr, g * 64 + np.arange(DI), g * 64 + np.arange(DI)] = \
                    ii["Dskip"][gg, dr]
        w["dskW"] = _bf(dskWm)
        w.update(winTu=winTu, winTz=winTz, conv4T=_bf(conv4T), convb=convb,
                 dtWT=_bf(dtWT), dtb=dtb, xprojBCT=_bf(xprojBCT), asc=asc,
                 dsk=dsk, woutT=_bf(woutT))
        maps_w.append(w)

    in_maps = []
    for k in range(NCORE):
        b, gs = k // 2, k % 2
        m = dict(maps_w[gs])
        xp = np.zeros((C, 66, 66), np.float32)
        xp[:, 1:65, 1:65] = x[b]
        m["xpad"] = np.ascontiguousarray(xp.reshape(C, 66 * 66))
        in_maps.append(m)
    return in_maps


_CACHE = {}


def kernel(**inputs):
    from concourse.bass_utils import run_bass_kernel_spmd
    if "nc" not in _CACHE:
        _CACHE["nc"] = _build_nc()
    nc = _CACHE["nc"]
    in_maps = _prep_inputs(inputs)
    res = run_bass_kernel_spmd(nc, in_maps, list(range(NCORE))).results
    out = np.stack([np.asarray(res[2 * b]["outp"]).reshape(OUT, H, W)
                    for b in range(B)])
    return out.astype(np.float32)
